# revision 1
# baseline (speedup 1.0000x reference)
"""Trainium2 Bass kernel for nn_Attention_5815385719367 (gnn_message_passing).

Computation (see reference):
  map_q/k/v = map_code @ Wq/Wk/Wv ; obs_k/v = obs_code @ Wk/Wv
  scores    = [sum(q*k,-1) | q @ obs_k.T] / 8
  w         = softmax(scores)
  agg       = w[:, :1]*glu(map_v) + w[:, 1:] @ glu(obs_v)
  out       = LN(agg @ Wo + bo + map_code) * gamma + beta

Sharding: data-parallel over N_map rows (2048 rows/core x 8 cores);
obs_code and weights replicated. No collectives.

v2 design notes (per core):
  - scores computed TRANSPOSED in PSUM: ST[obs=128, map] via PE ROW-TILED
    pairs: even obs block's k.T sits on SBUF partitions 0-63 (PE tile
    (0,0)), odd block's on partitions 64-127 (tile (64,0)); the two
    256-cycle streams run CONCURRENTLY in different PE row groups, so
    ST costs ~0.5 cyc/col. qT and okT are produced pre-duplicated /
    parity-split by projecting with host-duplicated weights
    ([64,128] wq|wq and wk|wk), so no cross-partition copies exist.
  - softmax exp is the hard wall (1 elem/lane/cycle on ACT): split it
    between ACT (direct exp -> fp8 e5m2, logits shifted by +2 so the
    whole empirical logit range [-8.1, 8.32] fits e5m2 with no
    clipping) and DVE (Schraudolph: one mult-add tensor_scalar into
    uint8, whose bit pattern IS the e5m2 log-domain approximation;
    errors ~+-10% are noise-like and average out over 8k obs). The
    PE consumes exp'd scores one pair late, so the PE FIFO never
    head-of-line blocks on an exp in flight.
  - PV runs fp8 DoubleRow over block PAIRS: stationary
    gob8[128, 2, 66] = glu(obs_v)|ones|zero-pad for (even, odd)
    blocks, moving pt8[128, 2, 512] = exp'd scores; contraction is
    256 at 0.5 cyc/col. M padded 33->66 keeps col tiling off
    (DoubleRow is incompatible with column tiling). The ones column
    accumulates the softmax denominator for free.
  - self-attention term folded into the PV accumulator seed: an
    identity-33 stationary matmul deposits selfexp*glu(map_v).T into
    agg rows 0-31 and selfexp into the denominator row, so the
    epilogue is just (agg @ Wo)/denom + map + bo -> LayerNorm; the
    logit shift cancels exactly in the ratio.
  - projections in bf16 (inputs shipped bf16), epilogue Wo in bf16.
  - GPSIMD does SBUF-side elementwise work (sigmoid affine steps,
    map+bo, gamma/beta) since it cannot touch PSUM.
"""

import numpy as np

import concourse.bass as bass
import concourse.bacc as bacc
import concourse.tile as tile
from concourse import mybir
from concourse.bass_utils import run_bass_kernel_spmd

NCORES = 8
NM, NO, E = 16384, 8192, 64
NS = NM // NCORES            # 2048 map rows per core
H = E // 2                   # 32
TEMP = 8.0
EPS = 1e-6
P = 128
NT = NS // P                 # 16 row tiles per core
GW = 512                     # map group width (psum bank)
NPAIR = NO // 256            # 32 obs block-pairs
SHIFT = -2.0                 # logit shift: exp(l - SHIFT), cancels in ratio.
                             # Logits empirically span [-8.1, 8.32]; e5m2's
                             # 22-e-fold range with shift -2 covers all of it
                             # (max exp ~ e^10.3 = 3e4 < 57344) with no
                             # clipping at either end.
MPAD = 80                    # padded PV output partitions (33 real; %16 keeps the DoubleRow ldweights step legal, >64 keeps column tiling off)

F32 = mybir.dt.float32
F32R = mybir.dt.float32r
BF16 = mybir.dt.bfloat16
FP8 = mybir.dt.float8e4
FP8E5 = mybir.dt.float8e5
U8 = mybir.dt.uint8
AF = mybir.ActivationFunctionType
ALU = mybir.AluOpType
DR = mybir.MatmulPerfMode.DoubleRow

# Schraudolph constants for uint8 e5m2 log-domain exp of RAW score s:
#   i = 4*log2(exp(s/8 - SHIFT)) + 60 - sawtooth_center
# DVE float->uint8 conversion saturates [0,255] and rounds RNE (probed).
SCH_A = 4.0 * 1.4426950408889634 / TEMP   # 0.72135
SCH_B = 60.0 - 4.0 * 1.4426950408889634 * SHIFT - 0.229

# layout of the bf16 weight pack [64, BW]
_WQ0 = 0              # wq duplicated [64, 128]
_WK0 = 128            # wk duplicated [64, 128]
_WV0 = 256            # wv [64, 64]
BW = 320

# layout of the f32r pack [64, FW]
_ONES0 = 0            # ones column [64, 1]
_IDO = 1              # identity [33, MPAD] seed stationary
_WOE0 = 1 + MPAD      # woe [33 rows used, 66]
FW = 1 + MPAD + E + 2


def _bc_part(ap, n):
    """Broadcast a [x, ...] AP along a new leading partition dim of n."""
    return bass.AP(tensor=ap.tensor, offset=ap.offset, ap=[[0, n]] + list(ap.ap))


def _emit(tc, out_d, map_rows_d, mapT_d, obsT_d, wpb_d, wpf_d, c8_d, vec_d,
          dbg=None, exp_act_frac=0.5):
    nc = tc.nc
    with tc.tile_pool(name="consts", bufs=1) as consts, \
         tc.tile_pool(name="big", bufs=1) as big, \
         tc.tile_pool(name="sb_sm", bufs=3) as sb_sm, \
         tc.tile_pool(name="sb_pt", bufs=4) as sb_pt, \
         tc.tile_pool(name="ps", bufs=3, space="PSUM") as ps, \
         tc.tile_pool(name="ps_agg", bufs=2, space="PSUM") as ps_agg:

        # ---------------- constants ----------------
        wpb = consts.tile([E, BW], BF16)          # bf16 weights pack
        nc.sync.dma_start(wpb, wpb_d)
        wq2 = wpb[:, _WQ0:_WQ0 + 128]             # [64,128] wq|wq
        wk2 = wpb[:, _WK0:_WK0 + 128]             # [64,128] wk|wk
        wv = wpb[:, _WV0:_WV0 + E]                # [64,64]

        wpf = consts.tile([E, FW], F32R)
        nc.sync.dma_start(wpf, wpf_d)
        ones64 = wpf[:, _ONES0:_ONES0 + 1]
        id33 = wpf[0:H + 1, _IDO:_IDO + MPAD]     # identity seed [33, 80]
        woe = wpf[0:H + 1, _WOE0:_WOE0 + E + 2]   # [33,66]

        vecs = consts.tile([P, 3 * E + 1], F32)   # bo|gamma|beta|-shift
        nc.sync.dma_start(vecs, _bc_part(vec_d, P))
        bo_b = vecs[:, 0:E]
        ga_b = vecs[:, E:2 * E]
        be_b = vecs[:, 2 * E:3 * E]
        msh = vecs[:, 3 * E:3 * E + 1]            # -SHIFT bias column

        # ---------------- big arenas + input DMAs ----------------
        mapT = big.tile([E, NS], BF16)
        obsT = big.tile([E, NO], BF16)
        for lo, hi, t_, s_ in ((0, 512, mapT, mapT_d),
                               (0, 1024, obsT, obsT_d),
                               (512, 1024, mapT, mapT_d),
                               (1024, 2048, obsT, obsT_d),
                               (1024, 2048, mapT, mapT_d),
                               (2048, 4096, obsT, obsT_d),
                               (4096, 8192, obsT, obsT_d)):
            nc.sync.dma_start(t_[:, lo:hi], s_[:, lo:hi])
        map_rows = big.tile([P, NT, E], F32)
        nc.sync.dma_start(map_rows, map_rows_d.rearrange("(t p) e -> p t e", p=P))

        qT = big.tile([P, NS], F32R)              # map_q.T duplicated halves
        gmT = big.tile([H + 1, NS], F32R)         # [glu(map_v).T ; selfexp]
        okT = big.tile([P, NPAIR, P], F32R)       # obs_k.T parity-split
        gob8 = big.tile([P, NPAIR, 2, MPAD], FP8)  # glu(obs_v)|1|0 pairs
        ags = big.tile([H + 1, NS], F32R)         # [numer.T ; denom]
        map_pb = big.tile([P, NT, E], F32)        # map + bo
        out_pre = big.tile([P, NT, E], F32)
        out_all = big.tile([P, NT, E], F32)
        mvC = big.tile([P, NT, 2], F32)
        rstd = big.tile([P, NT], F32)

        # gob8 static columns: ones at h=32, zeros at h=33..65 (DMA from
        # the small HBM consts tensor, replicated via zero strides)
        gob8f = gob8.rearrange("p a b c -> p (a b) c")
        C8W = MPAD - H
        ones_src = bass.AP(tensor=c8_d.tensor, offset=c8_d.offset,
                           ap=[[C8W, P], [0, 2 * NPAIR], [0, 1]])
        nc.sync.dma_start(gob8f[:, :, H:H + 1], ones_src)
        zero_src = bass.AP(tensor=c8_d.tensor, offset=c8_d.offset + 1,
                           ap=[[C8W, P], [0, 2 * NPAIR], [1, MPAD - H - 1]])
        nc.sync.dma_start(gob8f[:, :, H + 1:MPAD], zero_src)

        # map + bo on gpsimd (all-SBUF)
        bo_rep = bass.AP(tensor=bo_b.tensor, offset=bo_b.offset,
                         ap=[list(bo_b.ap[0]), [0, NT], [1, E]])
        nc.gpsimd.tensor_tensor(out=map_pb, in0=map_rows, in1=bo_rep,
                                op=ALU.add)

        # ---------------- prologue pieces ----------------
        def map_chunk(c):
            """q (duplicated), selfexp, glu(map_v) for map cols [c*512, ..)."""
            sl = slice(c * GW, (c + 1) * GW)
            q_ps = ps.tile([P, 2, GW], F32, tag="st", name=f"qps{c}")
            nc.tensor.matmul(q_ps[:, 0, :], wq2, mapT[:, sl],
                             start=True, stop=True)
            nc.vector.tensor_copy(qT[:, sl], q_ps[:, 0, :])
            k_ps = ps.tile([P, 2, GW], F32, tag="st", name=f"kps{c}")
            nc.tensor.matmul(k_ps[:, 0, :], wk2, mapT[:, sl],
                             start=True, stop=True)
            qk = sb_sm.tile([E, GW], F32R, tag="qk", name=f"qk{c}")
            nc.vector.tensor_tensor(out=qk, in0=qT[0:E, sl],
                                    in1=k_ps[0:E, 0, :], op=ALU.mult)
            # self-score sum lands in the unused upper half of k_ps
            ss_ps = k_ps[0:1, 1, :]
            nc.tensor.matmul(ss_ps, ones64, qk, start=True, stop=True)
            nc.scalar.activation(gmT[H:H + 1, sl], ss_ps, AF.Exp,
                                 scale=1.0 / TEMP, bias=msh[0:1])
            v_ps = ps.tile([P, 2, GW], F32, tag="st", name=f"vps{c}")
            nc.tensor.matmul(v_ps[0:E, 0, :], wv, mapT[:, sl],
                             start=True, stop=True)
            th = sb_sm.tile([H, GW], F32, tag="th", name=f"th{c}")
            nc.scalar.activation(th, v_ps[H:E, 0, :], AF.Tanh, scale=0.5)
            nc.gpsimd.tensor_scalar(out=th, in0=th, scalar1=0.5, scalar2=0.5,
                                    op0=ALU.mult, op1=ALU.add)
            nc.vector.tensor_tensor(out=gmT[0:H, sl], in0=v_ps[0:H, 0, :],
                                    in1=th, op=ALU.mult)

        def obs_k_chunk2(c2, eng="v"):
            """okT parity-split fill for TWO obs chunks (one PSUM alloc).

            chunk c covers obs cols [c*512, ..) = blocks 4c..4c+3; even
            blocks land on partitions 0-63 of okT, odd blocks on 64-127
            (via the duplicated upper half of the wk2 projection, so no
            cross-partition movement is needed)."""
            k_ps = ps.tile([P, 2, GW], F32, tag="st", name=f"okps{c2}")
            for t in range(2):
                c = 2 * c2 + t
                sl = slice(c * GW, (c + 1) * GW)
                nc.tensor.matmul(k_ps[:, t, :], wk2, obsT[:, sl],
                                 start=True, stop=True)
            for t in range(2):
                c = 2 * c2 + t
                ev_in = k_ps[0:E, t, :].rearrange("p (b m) -> p b m",
                                                  b=2)[:, :, 0:P]
                ev_out = okT[0:E, 2 * c:2 * c + 2, :]
                od_in = k_ps[E:P, t, :].rearrange("p (b m) -> p b m",
                                                  b=2)[:, :, P:2 * P]
                od_out = okT[E:P, 2 * c:2 * c + 2, :]
                if eng == "v":
                    nc.vector.tensor_copy(ev_out, ev_in)
                    nc.vector.tensor_copy(od_out, od_in)
                else:
                    nc.scalar.copy(ev_out, ev_in)
                    nc.scalar.copy(od_out, od_in)

        def obs_v_batch2(c2, nb=16):
            """glu(obs_v) for nb consecutive obs blocks (one PSUM alloc)."""
            v_ps = ps.tile([P, 16, E], F32, tag="st", name=f"ovps{c2}")
            for b in range(nb):
                blk = c2 * 16 + b
                nc.tensor.matmul(v_ps[:, b, :],
                                 obsT[:, blk * P:(blk + 1) * P], wv,
                                 start=True, stop=True)
            tho = sb_sm.tile([P, 16, H], F32, tag="tho", name=f"tho{c2}")
            nc.scalar.activation(tho[:, 0:nb, :], v_ps[:, 0:nb, H:E],
                                 AF.Tanh, scale=0.5)
            nc.gpsimd.tensor_scalar(out=tho[:, 0:nb, :], in0=tho[:, 0:nb, :],
                                    scalar1=0.5, scalar2=0.5,
                                    op0=ALU.mult, op1=ALU.add)
            # blocks 16*c2.. -> pairs 8*c2.., t = parity
            og = gob8[:, 8 * c2:8 * c2 + nb // 2, :, 0:H]
            vi = v_ps[:, 0:nb, 0:H].rearrange("p (a b) h -> p a b h", b=2)
            ti = tho[:, 0:nb, :].rearrange("p (a b) h -> p a b h", b=2)
            nc.vector.tensor_tensor(out=og, in0=vi, in1=ti, op=ALU.mult)

        def agg_flush(g, agg, eng="v"):
            sl = slice(g * GW, (g + 1) * GW)
            if eng == "v":
                nc.vector.tensor_copy(ags[0:H + 1, sl], agg[0:H + 1, :])
            else:
                nc.scalar.copy(ags[0:H + 1, sl], agg[0:H + 1, :])

        # ---------------- epilogue ----------------
        def epi_half(half):
            """Batched epilogue for 8 map tiles: all PE matmuls first
            (into two grouped PSUM tiles, 512B-strided so every [128,66]
            output stays within one bank), then the elementwise chain
            pipelines across tiles on DVE/ACT without PE round-trips."""
            base = half * (NT // 2)
            uda = ps.tile([P, 8, P], F32, tag="st", name=f"uda{half}")
            for i in range(8):
                sl = slice((base + i) * P, (base + i + 1) * P)
                nc.tensor.matmul(uda[:, i, 0:E + 2], ags[:, sl], woe,
                                 start=True, stop=True)
            # evacuate PSUM immediately (uda sits in the ST rotation --
            # holding it through the elementwise chain would strangle the
            # main loop when this runs inside hp1)
            uds = sb_sm.tile([P, 8, E + 2], F32, tag="uds", name=f"uds{half}")
            nc.vector.tensor_copy(uds, uda[:, :, 0:E + 2])
            rden = sb_sm.tile([P, 8], F32, tag="rden", name=f"rden{half}")
            nc.vector.reciprocal(rden, uds[:, :, E])
            for i in range(8):
                t = base + i
                nc.vector.scalar_tensor_tensor(out=out_pre[:, t, :],
                                               in0=uds[:, i, 0:E],
                                               scalar=rden[:, i:i + 1],
                                               in1=map_pb[:, t, :],
                                               op0=ALU.mult, op1=ALU.add)
                stats = sb_sm.tile([P, 6], F32, tag="stats", name=f"stats{t}")
                nc.vector.bn_stats(stats, out_pre[:, t, :])
                nc.vector.bn_aggr(mvC[:, t, :], stats)

        def epi_final(half, act_assist=True):
            tsl = slice(half * (NT // 2), (half + 1) * (NT // 2))
            w = NT // 2
            vpe = sb_sm.tile([P, w], F32, tag="vpe", name=f"vpe{half}")
            nc.vector.tensor_scalar_add(vpe, mvC[:, tsl, 1], EPS)
            c1 = sb_sm.tile([P, w], F32, tag="nc1", name=f"nc1{half}")
            nc.vector.tensor_scalar(out=c1, in0=vpe, scalar1=0.564185,
                                    scalar2=0.378467, op0=ALU.mult,
                                    op1=ALU.add)
            c2 = sb_sm.tile([P, w], F32, tag="nc2", name=f"nc2{half}")
            nc.vector.tensor_scalar(out=c2, in0=vpe, scalar1=0.288949,
                                    scalar2=0.791321, op0=ALU.mult,
                                    op1=ALU.add)
            nc.vector.tensor_tensor(out=c1, in0=c1, in1=c2, op=ALU.min)
            rs = rstd[:, tsl]
            nc.vector.reciprocal(rs, c1)
            for _ in range(3):
                nc.vector.tensor_tensor(out=c1, in0=rs, in1=rs, op=ALU.mult)
                nc.vector.tensor_tensor(out=c1, in0=c1, in1=vpe, op=ALU.mult)
                nc.vector.tensor_scalar(out=c1, in0=c1, scalar1=-0.5,
                                        scalar2=1.5, op0=ALU.mult,
                                        op1=ALU.add)
                nc.vector.tensor_tensor(out=rs, in0=rs, in1=c1, op=ALU.mult)
            for t in range(half * (NT // 2), (half + 1) * (NT // 2)):
                xn = sb_sm.tile([P, E], F32, tag="xn", name=f"xn{t}")
                if act_assist:
                    nmr = sb_sm.tile([P, 1], F32, tag="nmr", name=f"nmr{t}")
                    nc.vector.tensor_scalar(out=nmr, in0=mvC[:, t, 0:1],
                                            scalar1=rstd[:, t:t + 1],
                                            scalar2=-1.0, op0=ALU.mult,
                                            op1=ALU.mult)
                    nc.scalar.activation(xn, out_pre[:, t, :], AF.Identity,
                                         bias=nmr, scale=rstd[:, t:t + 1])
                else:
                    nc.vector.tensor_scalar(out=xn, in0=out_pre[:, t, :],
                                            scalar1=mvC[:, t, 0:1],
                                            scalar2=rstd[:, t:t + 1],
                                            op0=ALU.subtract, op1=ALU.mult)
                nc.gpsimd.tensor_tensor(out=xn, in0=xn, in1=ga_b, op=ALU.mult)
                nc.gpsimd.tensor_tensor(out=out_all[:, t, :], in0=xn,
                                        in1=be_b, op=ALU.add)
            od = out_d.rearrange("(t p) e -> p t e", p=P)
            for q in range(2):
                qsl = slice(half * (NT // 2) + q * (NT // 4),
                            half * (NT // 2) + (q + 1) * (NT // 4))
                nc.sync.dma_start(od[:, qsl, :], out_all[:, qsl, :])

        # ---------------- prologue head ----------------
        # The self-attention fold needs gmT rows 0-31 scaled by the
        # selfexp row (per map column). Engines cannot partition-broadcast
        # and neither can SBUF-source DMAs, so bounce each chunk's selfexp
        # slice through HBM and back with a broadcast DRAM read; chunk-wise
        # so the round-trip latency hides behind later prologue work.
        sxp_hbm = nc.dram_tensor("sxp_hbm", [NS], F32R, kind="Internal").ap()
        sxp = big.tile([H, NS], F32R)

        def sxp_fold(c):
            sl = slice(c * GW, (c + 1) * GW)
            nc.sync.dma_start(sxp_hbm[sl], gmT[H:H + 1, sl])
            nc.sync.dma_start(sxp[:, sl], _bc_part(sxp_hbm[sl], H))
            nc.vector.tensor_tensor(out=gmT[0:H, sl], in0=gmT[0:H, sl],
                                    in1=sxp[:, sl], op=ALU.mult)

        map_chunk(0)
        map_chunk(1)
        sxp_fold(0)
        obs_k_chunk2(0)
        sxp_fold(1)
        obs_v_batch2(0)
        map_chunk(2)
        sxp_fold(2)
        map_chunk(3)
        sxp_fold(3)

        # drip the remaining prologue into the first half-pass
        # drip schedule. IMPORTANT: obs_v_batch stays a single drip unit --
        # its PSUM tile comes from the shared rotating "st" tag, so the glu
        # must read it before the main loop's next st allocations wrap
        # around the pool and clobber the bank.
        drip = {}
        items = []
        for c2 in range(1, NO // GW // 2):
            items.append((4 * (c2 - 1), lambda c2=c2: obs_k_chunk2(c2)))
        for b2 in range(1, 4):
            items.append((7 * b2 - 3, lambda b2=b2: obs_v_batch2(b2)))
        items.sort(key=lambda x: x[0])
        used = set()
        for want, fn in items:
            pp = want
            while pp in used:
                pp += 1
            used.add(pp)
            drip.setdefault(pp, []).append(fn)

        # exp unit assignment: alternate engines per (pair, group); bias
        # toward ACT by granting it both groups every few pairs.
        def exp_unit(st_t, pt_t, eng):
            if eng == "a":
                nc.scalar.activation(pt_t, st_t, AF.Exp,
                                     scale=1.0 / TEMP, bias=msh)
            else:
                nc.vector.tensor_scalar(out=pt_t.bitcast(U8), in0=st_t,
                                        scalar1=SCH_A, scalar2=SCH_B,
                                        op0=ALU.mult, op1=ALU.add)

        # ---------------- main loop: 2 half-passes x 32 pairs ----------
        # Software-pipelined by one pair: the PV for pair p-1 is issued to
        # the PE AFTER pair p's ST matmuls, so by the time the PE FIFO
        # reaches it, exp(p-1) has long finished -- no head-of-line stall.
        for hp in range(2):
            agg0 = ps_agg.tile([MPAD, GW], F32, tag="agg", name=f"agg{hp}_0")
            agg1 = ps_agg.tile([MPAD, GW], F32, tag="agg", name=f"agg{hp}_1")
            g0 = 2 * hp
            g1 = 2 * hp + 1
            s0 = slice(g0 * GW, (g0 + 1) * GW)
            s1 = slice(g1 * GW, (g1 + 1) * GW)
            nc.tensor.matmul(agg0, id33, gmT[:, s0],
                             start=True, stop=False)
            nc.tensor.matmul(agg1, id33, gmT[:, s1],
                             start=True, stop=False)
            prev_pt = None
            for pp in range(NPAIR):
                st0 = ps.tile([P, 2, GW], F32, tag="st", name=f"st{hp}_{pp}_0")
                st1 = ps.tile([P, 2, GW], F32, tag="st", name=f"st{hp}_{pp}_1")
                ko_lo = okT[0:E, pp, :]
                ko_hi = okT[E:P, pp, :]
                nc.tensor.matmul(st0[:, 0, :], ko_lo, qT[0:E, s0],
                                 start=True, stop=True)
                nc.tensor.matmul(st0[:, 1, :], ko_hi, qT[E:P, s0],
                                 start=True, stop=True)
                nc.tensor.matmul(st1[:, 0, :], ko_lo, qT[0:E, s1],
                                 start=True, stop=True)
                nc.tensor.matmul(st1[:, 1, :], ko_hi, qT[E:P, s1],
                                 start=True, stop=True)
                if prev_pt is not None:
                    qq, qt0, qt1 = prev_pt
                    go = gob8[:, qq, :, :]
                    nc.tensor.matmul(agg0, go, qt0, start=False, stop=False,
                                     perf_mode=DR)
                    nc.tensor.matmul(agg1, go, qt1, start=False, stop=False,
                                     perf_mode=DR)
                pt0 = sb_pt.tile([P, 2, GW], FP8E5, tag="pt",
                                 name=f"pt{hp}_{pp}_0")
                pt1 = sb_pt.tile([P, 2, GW], FP8E5, tag="pt",
                                 name=f"pt{hp}_{pp}_1")
                # exp split: in hp0 DVE also carries the drip (casts/glu),
                # so ACT takes both groups every 4th pair; in hp1 the
                # engines are evenly loaded, so strict 1:1.
                bonus = (pp % 4 == 3) if hp == 0 else False
                exp_unit(st0, pt0, "a")
                exp_unit(st1, pt1, "a" if bonus else "v")
                prev_pt = (pp, pt0, pt1)
                if hp == 0:
                    for fn in drip.get(pp, ()):
                        fn()
                else:
                    # hp0's ags columns are final: run its epilogue during
                    # hp1 (batched -- only two extra PSUM allocs total)
                    if pp == 6:
                        epi_half(0)
                    elif pp == 16:
                        epi_final(0)
            qq, qt0, qt1 = prev_pt
            go = gob8[:, qq, :, :]
            nc.tensor.matmul(agg0, go, qt0, start=False, stop=True,
                             perf_mode=DR)
            nc.tensor.matmul(agg1, go, qt1, start=False, stop=True,
                             perf_mode=DR)
            agg_flush(g0, agg0, eng="v")
            agg_flush(g1, agg1, eng="a")

        # ---------------- epilogue (half 0 ran during hp1) ----------
        epi_half(1)
        epi_final(1)

        if dbg is not None:
            nc.sync.dma_start(dbg["qT"], qT)
            nc.sync.dma_start(dbg["gmT"], gmT)
            nc.sync.dma_start(dbg["ags"], ags)
            nc.sync.dma_start(dbg["okT"], okT.rearrange("p a b -> p (a b)"))
            nc.sync.dma_start(dbg["gob8"],
                              gob8.rearrange("p a b c -> p (a b c)"))
            nc.sync.dma_start(dbg["out_pre"],
                              out_pre.rearrange("p a b -> p (a b)"))
            nc.sync.dma_start(dbg["mvC"], mvC.rearrange("p a b -> p (a b)"))


_CACHED = None


def _build(debug=False):
    global _CACHED
    if _CACHED is not None and not debug:
        return _CACHED
    nc = bacc.Bacc("TRN2", target_bir_lowering=False, debug=False)

    def din(name, shape, dt=F32):
        return nc.dram_tensor(name, shape, dt, kind="ExternalInput").ap()

    map_rows_d = din("map_rows", [NS, E])
    mapT_d = din("mapT", [E, NS], BF16)
    obsT_d = din("obsT", [E, NO], BF16)
    wpb_d = din("wpb", [E, BW], BF16)
    wpf_d = din("wpf", [E, FW], F32R)
    c8_d = din("c8", [P, MPAD - H], FP8)
    vec_d = din("vpack", [3 * E + 1])
    out_d = nc.dram_tensor("out", [NS, E], F32, kind="ExternalOutput").ap()

    dbg = None
    if debug:
        def dout(name, shape, dt=F32):
            return nc.dram_tensor(name, shape, dt, kind="ExternalOutput").ap()
        dbg = {
            "qT": dout("dbg_qT", [P, NS], F32R),
            "gmT": dout("dbg_gmT", [H + 1, NS], F32R),
            "ags": dout("dbg_ags", [H + 1, NS], F32R),
            "okT": dout("dbg_okT", [P, NPAIR * P], F32R),
            "gob8": dout("dbg_gob8", [P, NPAIR * 2 * MPAD], FP8),
            "out_pre": dout("dbg_out_pre", [P, NT * E]),
            "mvC": dout("dbg_mvC", [P, NT * 2]),
        }

    with tile.TileContext(nc) as tc:
        _emit(tc, out_d, map_rows_d, mapT_d, obsT_d, wpb_d, wpf_d, c8_d,
              vec_d, dbg=dbg)
    nc.compile()
    if not debug:
        _CACHED = nc
    return nc


def _prep_in_maps(map_code, obs_code, Wq, Wk, Wv, Wo, bo, gamma, beta):
    f = np.float32
    map_code = np.ascontiguousarray(np.asarray(map_code, dtype=f))
    obs_code = np.asarray(obs_code, dtype=f)

    bf16_np = mybir.dt.np(BF16)
    fp8_np = mybir.dt.np(FP8)

    def to_bf16(x):
        return np.ascontiguousarray(np.asarray(x, dtype=f).astype(bf16_np))

    obsT = np.ascontiguousarray(obs_code.T)

    wq2 = np.concatenate([np.asarray(Wq, f), np.asarray(Wq, f)], axis=1)
    wk2 = np.concatenate([np.asarray(Wk, f), np.asarray(Wk, f)], axis=1)
    woe = np.zeros((E, E + 2), dtype=f)
    woe[0:H, 0:E] = np.asarray(Wo, dtype=f)
    woe[H, E] = 1.0
    wpb = np.zeros((E, BW), dtype=f)
    wpb[:, _WQ0:_WQ0 + 128] = wq2
    wpb[:, _WK0:_WK0 + 128] = wk2
    wpb[:, _WV0:_WV0 + E] = np.asarray(Wv, f)

    wpf = np.zeros((E, FW), dtype=f)
    wpf[:, _ONES0] = 1.0
    for k in range(H + 1):
        wpf[k, _IDO + k] = 1.0   # identity seed stationary [33, MPAD]
    wpf[:, _WOE0:_WOE0 + E + 2] = woe

    c8 = np.zeros((P, MPAD - H), dtype=fp8_np)
    c8[:, 0] = 1.0

    vpack = np.concatenate([
        np.asarray(bo, dtype=f), np.asarray(gamma, dtype=f),
        np.asarray(beta, dtype=f), np.full((1,), -SHIFT, dtype=f),
    ])
    shared = {
        "obsT": to_bf16(obsT),
        "wpb": to_bf16(wpb),
        "wpf": np.ascontiguousarray(wpf),
        "c8": np.ascontiguousarray(c8),
        "vpack": np.ascontiguousarray(vpack),
    }
    in_maps = []
    for i in range(NCORES):
        shard = map_code[i * NS:(i + 1) * NS]
        m = dict(shared)
        m["map_rows"] = shard
        m["mapT"] = to_bf16(np.ascontiguousarray(shard.T))
        in_maps.append(m)
    return in_maps


def run(trace=False, **inputs):
    nc = _build()
    in_maps = _prep_in_maps(**inputs)
    res = run_bass_kernel_spmd(nc, in_maps, list(range(NCORES)), trace=trace)
    out = np.concatenate([res.results[i]["out"] for i in range(NCORES)], axis=0)
    return out, res


def kernel(**inputs):
    out, _ = run(trace=False, **inputs)
    return out



# revision 2
# speedup vs baseline: 1.0763x; 1.0763x over previous
"""Trainium2 Bass kernel for nn_Attention_5815385719367 (gnn_message_passing).

Computation (see reference):
  map_q/k/v = map_code @ Wq/Wk/Wv ; obs_k/v = obs_code @ Wk/Wv
  scores    = [sum(q*k,-1) | q @ obs_k.T] / 8
  w         = softmax(scores)
  agg       = w[:, :1]*glu(map_v) + w[:, 1:] @ glu(obs_v)
  out       = LN(agg @ Wo + bo + map_code) * gamma + beta

Sharding: data-parallel over N_map rows (2048 rows/core x 8 cores);
obs_code and weights replicated. No collectives.

v3 design notes (per core), building on v2:
  - scores computed TRANSPOSED in PSUM: ST[obs=128, map] via PE ROW-TILED
    pairs: even obs block's k.T sits on SBUF partitions 0-63 (PE tile
    (0,0)), odd block's on partitions 64-127 (tile (64,0)).
  - v3: qT/okS/gmT/ones/id33 are BF16 (v2 used f32r). f32 moving
    operands stream the PE at 2 cyc/col; bf16 streams 1 cyc/col, so the
    ST matmuls halve (427 -> ~220 ns each measured).
  - v3: okS keeps the k.T projection in its natural [128, NO] layout
    (dup halves from the wk|wk projection); the even/odd parity split
    is done by SLICING (partitions 0:64 x even cols / 64:128 x odd
    cols) -- v2's per-block parity copies were pure waste.
  - v3: a ~4us junk-matmul warmup burst runs during the input DMA so
    the PE HAM clock-gate opens (1.2 -> 2.4 GHz) before real work; v2
    ran the whole prologue + 10 pairs cold.
  - v3: lean prologue -- only map chunks 0,1 (hp0's groups), okS chunk
    pair 0 and gob batch 0 precede the main loop; map chunks 2,3 (only
    needed by hp1) and the rest of okS/gob drip into hp0.
  - softmax exp is the hard wall (1 elem/lane/cycle on ACT): split it
    between ACT (direct exp -> fp8 e5m2, logits shifted by +2) and DVE
    (Schraudolph: one mult-add tensor_scalar into uint8 whose bit
    pattern IS the e5m2 log-domain approximation).
  - PV runs fp8 DoubleRow over block PAIRS: stationary
    gob8[128, 2, 80] = glu(obs_v)|ones|zero-pad, moving pt8; the ones
    column accumulates the softmax denominator for free.
  - self-attention term folded into the PV accumulator seed via an
    identity-33 stationary matmul.
  - v3: tail epilogue (half 1) uses DVE for the normalize step and two
    big broadcast tensor_tensor ops for gamma/beta instead of 16
    small per-tile ops.
"""

import numpy as np

import concourse.bass as bass
import concourse.bacc as bacc
import concourse.tile as tile
from concourse import mybir
from concourse.bass_utils import run_bass_kernel_spmd

NCORES = 8
NM, NO, E = 16384, 8192, 64
NS = NM // NCORES            # 2048 map rows per core
H = E // 2                   # 32
TEMP = 8.0
EPS = 1e-6
P = 128
NT = NS // P                 # 16 row tiles per core
GW = 512                     # map group width (psum bank)
NPAIR = NO // 256            # 32 obs block-pairs
SHIFT = -2.0                 # logit shift: exp(l - SHIFT), cancels in ratio.
MPAD = 80                    # padded PV output partitions (33 real; %16 keeps
                             # the DoubleRow ldweights step legal, >64 keeps
                             # column tiling off)

F32 = mybir.dt.float32
F32R = mybir.dt.float32r
BF16 = mybir.dt.bfloat16
FP8 = mybir.dt.float8e4
FP8E5 = mybir.dt.float8e5
U8 = mybir.dt.uint8
AF = mybir.ActivationFunctionType
ALU = mybir.AluOpType
DR = mybir.MatmulPerfMode.DoubleRow

# Schraudolph constants for uint8 e5m2 log-domain exp of RAW score s:
#   i = 4*log2(exp(s/8 - SHIFT)) + 60 - sawtooth_center
SCH_A = 4.0 * 1.4426950408889634 / TEMP   # 0.72135
SCH_B = 60.0 - 4.0 * 1.4426950408889634 * SHIFT - 0.229

# layout of the bf16 weight pack [64, BW]
_WQ0 = 0              # wq duplicated [64, 128]
_WK0 = 128            # wk duplicated [64, 128]
_WV0 = 256            # wv [64, 64]
_ONES0 = 320          # ones column [64, 1]
_IDO = 321            # identity [33, MPAD] seed stationary
BW = 321 + MPAD

# layout of the f32r pack [64, FW]: woe only
FW = E + 2


def _bc_part(ap, n):
    """Broadcast a [x, ...] AP along a new leading partition dim of n."""
    return bass.AP(tensor=ap.tensor, offset=ap.offset, ap=[[0, n]] + list(ap.ap))


def _emit(tc, out_d, map_rows_d, mapT_d, obsT_d, wpb_d, wpf_d, c8_d, vec_d,
          dbg=None):
    nc = tc.nc
    with tc.tile_pool(name="consts", bufs=1) as consts, \
         tc.tile_pool(name="big", bufs=1) as big, \
         tc.tile_pool(name="sb_sm", bufs=3) as sb_sm, \
         tc.tile_pool(name="sb_pt", bufs=4) as sb_pt, \
         tc.tile_pool(name="ps", bufs=3, space="PSUM") as ps, \
         tc.tile_pool(name="ps_agg", bufs=2, space="PSUM") as ps_agg:

        # ---------------- constants ----------------
        wpb = consts.tile([E, BW], BF16)          # bf16 weights pack
        nc.sync.dma_start(wpb, wpb_d)
        wq2 = wpb[:, _WQ0:_WQ0 + 128]             # [64,128] wq|wq
        wk2 = wpb[:, _WK0:_WK0 + 128]             # [64,128] wk|wk
        wv = wpb[:, _WV0:_WV0 + E]                # [64,64]
        ones64 = wpb[:, _ONES0:_ONES0 + 1]
        id33 = wpb[0:H + 1, _IDO:_IDO + MPAD]     # identity seed [33, 80]

        vecs = consts.tile([P, 3 * E + 1], F32)   # bo|gamma|beta|-shift
        nc.sync.dma_start(vecs, _bc_part(vec_d, P))
        bo_b = vecs[:, 0:E]
        ga_b = vecs[:, E:2 * E]
        be_b = vecs[:, 2 * E:3 * E]
        msh = vecs[:, 3 * E:3 * E + 1]            # -SHIFT bias column

        wpf = consts.tile([E, FW], F32R)
        nc.sync.dma_start(wpf, wpf_d)
        woe = wpf[0:H + 1, 0:E + 2]               # [33,66]

        # ---------------- warmup: open the HAM clock gate --------------
        # ~14 junk matmuls (~4us cold) on the weight pack while the input
        # DMAs land; output is scratch PSUM that is never read.
        for wi in range(14):
            wu = ps.tile([P, 2, GW], F32, tag="st", name=f"wu{wi}")
            nc.tensor.matmul(wu[:, 0, 0:320], wq2, wpb[:, 0:320],
                             start=True, stop=True)

        # ---------------- big arenas + input DMAs ----------------
        mapT = big.tile([E, NS], BF16)
        obsT = big.tile([E, NO], BF16)
        for lo, hi, t_, s_ in ((0, 512, mapT, mapT_d),
                               (0, 512, obsT, obsT_d),
                               (512, 1024, mapT, mapT_d),
                               (512, 1024, obsT, obsT_d),
                               (1024, 2048, obsT, obsT_d),
                               (1024, 2048, mapT, mapT_d),
                               (2048, 4096, obsT, obsT_d),
                               (4096, 8192, obsT, obsT_d)):
            nc.sync.dma_start(t_[:, lo:hi], s_[:, lo:hi])
        map_rows = big.tile([P, NT, E], F32)
        nc.sync.dma_start(map_rows, map_rows_d.rearrange("(t p) e -> p t e", p=P))

        qT = big.tile([P, NS], BF16)              # map_q.T duplicated halves
        gmT = big.tile([H + 1, NS], BF16)         # [glu(map_v).T ; selfexp]
        okS = big.tile([P, NO], BF16)             # obs_k.T duplicated halves
        gob8 = big.tile([P, NPAIR, 2, MPAD], FP8)  # glu(obs_v)|1|0 pairs
        ags = big.tile([H + 1, NS], F32R)         # [numer.T ; denom]
        map_pb = big.tile([P, NT, E], F32)        # map + bo
        out_pre = big.tile([P, NT, E], F32)
        out_all = big.tile([P, NT, E], F32)
        mvC = big.tile([P, NT, 2], F32)
        rstd = big.tile([P, NT], F32)

        # gob8 static columns: ones at h=32, zeros at h=33..79 (DMA from
        # the small HBM consts tensor, replicated via zero strides)
        gob8f = gob8.rearrange("p a b c -> p (a b) c")
        C8W = MPAD - H
        ones_src = bass.AP(tensor=c8_d.tensor, offset=c8_d.offset,
                           ap=[[C8W, P], [0, 2 * NPAIR], [0, 1]])
        nc.sync.dma_start(gob8f[:, :, H:H + 1], ones_src)
        zero_src = bass.AP(tensor=c8_d.tensor, offset=c8_d.offset + 1,
                           ap=[[C8W, P], [0, 2 * NPAIR], [1, MPAD - H - 1]])
        nc.sync.dma_start(gob8f[:, :, H + 1:MPAD], zero_src)

        # ---------------- prologue pieces ----------------
        # self-exp fold helper state: bounce selfexp through HBM for a
        # partition-broadcast read (engines cannot partition-broadcast).
        sxp_hbm = nc.dram_tensor("sxp_hbm", [NS], BF16, kind="Internal").ap()
        sxp = big.tile([H, NS], BF16)

        def map_chunk_a(c):
            """map_q (duplicated) for map cols [c*512, ..)."""
            sl = slice(c * GW, (c + 1) * GW)
            q_ps = ps.tile([P, 2, GW], F32, tag="st", name=f"qps{c}")
            nc.tensor.matmul(q_ps[:, 0, :], wq2, mapT[:, sl],
                             start=True, stop=True)
            nc.vector.tensor_copy(qT[:, sl], q_ps[:, 0, :])

        def map_chunk_b(c):
            """map_k, self-score, selfexp for map cols [c*512, ..)."""
            sl = slice(c * GW, (c + 1) * GW)
            k_ps = ps.tile([P, 2, GW], F32, tag="st", name=f"kps{c}")
            nc.tensor.matmul(k_ps[:, 0, :], wk2, mapT[:, sl],
                             start=True, stop=True)
            qk = sb_sm.tile([E, GW], BF16, tag="qk", name=f"qk{c}")
            nc.vector.tensor_tensor(out=qk, in0=qT[0:E, sl],
                                    in1=k_ps[0:E, 0, :], op=ALU.mult)
            # self-score sum lands in the unused upper half of k_ps
            ss_ps = k_ps[0:1, 1, :]
            nc.tensor.matmul(ss_ps, ones64, qk, start=True, stop=True)
            nc.scalar.activation(gmT[H:H + 1, sl], ss_ps, AF.Exp,
                                 scale=1.0 / TEMP, bias=msh[0:1])

        def map_chunk_c(c):
            """glu(map_v) for map cols [c*512, ..)."""
            sl = slice(c * GW, (c + 1) * GW)
            v_ps = ps.tile([P, 2, GW], F32, tag="st", name=f"vps{c}")
            nc.tensor.matmul(v_ps[0:E, 0, :], wv, mapT[:, sl],
                             start=True, stop=True)
            th = sb_sm.tile([H, GW], F32, tag="th", name=f"th{c}")
            nc.scalar.activation(th, v_ps[H:E, 0, :], AF.Tanh, scale=0.5)
            nc.gpsimd.tensor_scalar(out=th, in0=th, scalar1=0.5, scalar2=0.5,
                                    op0=ALU.mult, op1=ALU.add)
            nc.vector.tensor_tensor(out=gmT[0:H, sl], in0=v_ps[0:H, 0, :],
                                    in1=th, op=ALU.mult)

        def sxp_fold(c):
            sl = slice(c * GW, (c + 1) * GW)
            nc.sync.dma_start(sxp_hbm[sl], gmT[H:H + 1, sl])
            nc.sync.dma_start(sxp[:, sl], _bc_part(sxp_hbm[sl], H))
            nc.vector.tensor_tensor(out=gmT[0:H, sl], in0=gmT[0:H, sl],
                                    in1=sxp[:, sl], op=ALU.mult)

        def okS_chunk2(c2, eng="v"):
            """obs_k.T (dup halves) for obs cols [c2*1024, ..): 2 matmuls
            into one PSUM tile, ONE bulk cast out. No parity shuffling --
            the ST matmuls slice [0:64] x even cols / [64:128] x odd."""
            k_ps = ps.tile([P, 2, GW], F32, tag="st", name=f"okps{c2}")
            for t in range(2):
                c = 2 * c2 + t
                sl = slice(c * GW, (c + 1) * GW)
                nc.tensor.matmul(k_ps[:, t, :], wk2, obsT[:, sl],
                                 start=True, stop=True)
            dst = okS[:, c2 * 2 * GW:(c2 + 1) * 2 * GW]
            src = k_ps.rearrange("p a b -> p (a b)")
            if eng == "v":
                nc.vector.tensor_copy(dst, src)
            else:
                nc.scalar.copy(dst, src)

        def obs_v_batch2(c2, nb=16):
            """glu(obs_v) for nb consecutive obs blocks (one PSUM alloc)."""
            v_ps = ps.tile([P, 16, E], F32, tag="st", name=f"ovps{c2}")
            for b in range(nb):
                blk = c2 * 16 + b
                nc.tensor.matmul(v_ps[:, b, :],
                                 obsT[:, blk * P:(blk + 1) * P], wv,
                                 start=True, stop=True)
            tho = sb_sm.tile([P, 16, H], F32, tag="tho", name=f"tho{c2}")
            nc.scalar.activation(tho[:, 0:nb, :], v_ps[:, 0:nb, H:E],
                                 AF.Tanh, scale=0.5)
            nc.gpsimd.tensor_scalar(out=tho[:, 0:nb, :], in0=tho[:, 0:nb, :],
                                    scalar1=0.5, scalar2=0.5,
                                    op0=ALU.mult, op1=ALU.add)
            # blocks 16*c2.. -> pairs 8*c2.., t = parity
            og = gob8[:, 8 * c2:8 * c2 + nb // 2, :, 0:H]
            vi = v_ps[:, 0:nb, 0:H].rearrange("p (a b) h -> p a b h", b=2)
            ti = tho[:, 0:nb, :].rearrange("p (a b) h -> p a b h", b=2)
            nc.vector.tensor_tensor(out=og, in0=vi, in1=ti, op=ALU.mult)

        def map_pb_fill():
            bo_rep = bass.AP(tensor=bo_b.tensor, offset=bo_b.offset,
                             ap=[list(bo_b.ap[0]), [0, NT], [1, E]])
            nc.gpsimd.tensor_tensor(out=map_pb, in0=map_rows, in1=bo_rep,
                                    op=ALU.add)

        def agg_flush(g, agg, eng="v"):
            sl = slice(g * GW, (g + 1) * GW)
            if eng == "v":
                nc.vector.tensor_copy(ags[0:H + 1, sl], agg[0:H + 1, :])
            else:
                nc.scalar.copy(ags[0:H + 1, sl], agg[0:H + 1, :])

        # ---------------- epilogue ----------------
        def epi_half(half):
            """Batched epilogue for 8 map tiles: all PE matmuls first,
            then the elementwise chain pipelines on DVE."""
            base = half * (NT // 2)
            uda = ps.tile([P, 8, P], F32, tag="st", name=f"uda{half}")
            for i in range(8):
                sl = slice((base + i) * P, (base + i + 1) * P)
                nc.tensor.matmul(uda[:, i, 0:E + 2], ags[:, sl], woe,
                                 start=True, stop=True)
            uds = sb_sm.tile([P, 8, E + 2], F32, tag="uds", name=f"uds{half}")
            nc.vector.tensor_copy(uds, uda[:, :, 0:E + 2])
            rden = sb_sm.tile([P, 8], F32, tag="rden", name=f"rden{half}")
            nc.vector.reciprocal(rden, uds[:, :, E])
            for i in range(8):
                t = base + i
                nc.vector.scalar_tensor_tensor(out=out_pre[:, t, :],
                                               in0=uds[:, i, 0:E],
                                               scalar=rden[:, i:i + 1],
                                               in1=map_pb[:, t, :],
                                               op0=ALU.mult, op1=ALU.add)
                stats = sb_sm.tile([P, 6], F32, tag="stats", name=f"stats{t}")
                nc.vector.bn_stats(stats, out_pre[:, t, :])
                nc.vector.bn_aggr(mvC[:, t, :], stats)

        def epi_rstd(half):
            """1/sqrt(var+eps) for 8 tiles via min-poly + 3 NR steps."""
            tsl = slice(half * (NT // 2), (half + 1) * (NT // 2))
            w = NT // 2
            vpe = sb_sm.tile([P, w], F32, tag="vpe", name=f"vpe{half}")
            nc.vector.tensor_scalar_add(vpe, mvC[:, tsl, 1], EPS)
            c1 = sb_sm.tile([P, w], F32, tag="nc1", name=f"nc1{half}")
            nc.vector.tensor_scalar(out=c1, in0=vpe, scalar1=0.564185,
                                    scalar2=0.378467, op0=ALU.mult,
                                    op1=ALU.add)
            c2 = sb_sm.tile([P, w], F32, tag="nc2", name=f"nc2{half}")
            nc.vector.tensor_scalar(out=c2, in0=vpe, scalar1=0.288949,
                                    scalar2=0.791321, op0=ALU.mult,
                                    op1=ALU.add)
            nc.vector.tensor_tensor(out=c1, in0=c1, in1=c2, op=ALU.min)
            rs = rstd[:, tsl]
            nc.vector.reciprocal(rs, c1)
            for _ in range(3):
                nc.vector.tensor_tensor(out=c1, in0=rs, in1=rs, op=ALU.mult)
                nc.vector.tensor_tensor(out=c1, in0=c1, in1=vpe, op=ALU.mult)
                nc.vector.tensor_scalar(out=c1, in0=c1, scalar1=-0.5,
                                        scalar2=1.5, op0=ALU.mult,
                                        op1=ALU.add)
                nc.vector.tensor_tensor(out=rs, in0=rs, in1=c1, op=ALU.mult)

        def epi_final(half, act_assist=True):
            """Normalize + gamma/beta + output DMA for one half."""
            epi_rstd(half)
            t0, t1 = half * (NT // 2), (half + 1) * (NT // 2)
            for t in range(t0, t1):
                if act_assist:
                    # hp1-embedded: keep DVE free, use ACT + gpsimd
                    nmr = sb_sm.tile([P, 1], F32, tag="nmr", name=f"nmr{t}")
                    nc.vector.tensor_scalar(out=nmr, in0=mvC[:, t, 0:1],
                                            scalar1=rstd[:, t:t + 1],
                                            scalar2=-1.0, op0=ALU.mult,
                                            op1=ALU.mult)
                    xn = sb_sm.tile([P, E], F32, tag="xn", name=f"xn{t}")
                    nc.scalar.activation(xn, out_pre[:, t, :], AF.Identity,
                                         bias=nmr, scale=rstd[:, t:t + 1])
                    nc.gpsimd.tensor_tensor(out=xn, in0=xn, in1=ga_b,
                                            op=ALU.mult)
                    nc.gpsimd.tensor_tensor(out=out_all[:, t, :], in0=xn,
                                            in1=be_b, op=ALU.add)
                else:
                    # tail: DVE normalize into out_all
                    nc.vector.tensor_scalar(out=out_all[:, t, :],
                                            in0=out_pre[:, t, :],
                                            scalar1=mvC[:, t, 0:1],
                                            scalar2=rstd[:, t:t + 1],
                                            op0=ALU.subtract, op1=ALU.mult)
            if not act_assist:
                # two big broadcast ops over all 8 tiles at once
                ga_rep = bass.AP(tensor=ga_b.tensor, offset=ga_b.offset,
                                 ap=[list(ga_b.ap[0]), [0, t1 - t0], [1, E]])
                be_rep = bass.AP(tensor=be_b.tensor, offset=be_b.offset,
                                 ap=[list(be_b.ap[0]), [0, t1 - t0], [1, E]])
                nc.vector.tensor_tensor(out=out_all[:, t0:t1, :],
                                        in0=out_all[:, t0:t1, :],
                                        in1=ga_rep, op=ALU.mult)
                nc.vector.tensor_tensor(out=out_all[:, t0:t1, :],
                                        in0=out_all[:, t0:t1, :],
                                        in1=be_rep, op=ALU.add)
            od = out_d.rearrange("(t p) e -> p t e", p=P)
            for q in range(2):
                qsl = slice(t0 + q * (NT // 4), t0 + (q + 1) * (NT // 4))
                nc.sync.dma_start(od[:, qsl, :], out_all[:, qsl, :])

        # ---------------- prologue head (lean) ----------------
        # Only what hp0's first pairs need: map chunks 0,1 (group cols),
        # okS chunks 0,1 (pairs 0-3), gob batch 0 (pairs 0-7).
        map_chunk_a(0)
        map_chunk_b(0)
        map_chunk_c(0)
        sxp_fold(0)
        okS_chunk2(0)
        map_chunk_a(1)
        map_chunk_b(1)
        map_chunk_c(1)
        sxp_fold(1)
        obs_v_batch2(0)

        # drip schedule for hp0: item lists per pair index.
        # okS chunk c2 must land before pair 4*c2; gob batch b before
        # pair 8*b; map chunks 2,3 (qT/gmT for hp1) anywhere before hp1.
        drip = {
            0: [lambda: okS_chunk2(1)],
            2: [lambda: okS_chunk2(2)],
            4: [lambda: obs_v_batch2(1)],
            6: [lambda: okS_chunk2(3)],
            8: [lambda: okS_chunk2(4)],
            10: [lambda: obs_v_batch2(2)],
            12: [lambda: okS_chunk2(5), lambda: map_pb_fill()],
            14: [lambda: okS_chunk2(6)],
            16: [lambda: obs_v_batch2(3)],
            18: [lambda: okS_chunk2(7)],
            20: [lambda: map_chunk_a(2)],
            21: [lambda: map_chunk_b(2)],
            22: [lambda: map_chunk_c(2)],
            23: [lambda: sxp_fold(2)],
            24: [lambda: map_chunk_a(3)],
            25: [lambda: map_chunk_b(3)],
            26: [lambda: map_chunk_c(3)],
            27: [lambda: sxp_fold(3)],
        }

        def exp_unit(st_t, pt_t, eng):
            if eng == "a":
                nc.scalar.activation(pt_t, st_t, AF.Exp,
                                     scale=1.0 / TEMP, bias=msh)
            else:
                nc.vector.tensor_scalar(out=pt_t.bitcast(U8), in0=st_t,
                                        scalar1=SCH_A, scalar2=SCH_B,
                                        op0=ALU.mult, op1=ALU.add)

        # ---------------- main loop: 2 half-passes x 32 pairs ----------
        # Software-pipelined by one pair: the PV for pair p-1 is issued to
        # the PE AFTER pair p's ST matmuls, so by the time the PE FIFO
        # reaches it, exp(p-1) has long finished.
        for hp in range(2):
            agg0 = ps_agg.tile([MPAD, GW], F32, tag="agg", name=f"agg{hp}_0")
            agg1 = ps_agg.tile([MPAD, GW], F32, tag="agg", name=f"agg{hp}_1")
            g0 = 2 * hp
            g1 = 2 * hp + 1
            s0 = slice(g0 * GW, (g0 + 1) * GW)
            s1 = slice(g1 * GW, (g1 + 1) * GW)
            nc.tensor.matmul(agg0, id33, gmT[:, s0],
                             start=True, stop=False)
            nc.tensor.matmul(agg1, id33, gmT[:, s1],
                             start=True, stop=False)
            prev_pt = None
            for pp in range(NPAIR):
                co = 256 * pp
                ko_lo = okS[0:E, co:co + P]
                ko_hi = okS[E:P, co + P:co + 2 * P]
                st0 = ps.tile([P, 2, GW], F32, tag="st", name=f"st{hp}_{pp}_0")
                st1 = ps.tile([P, 2, GW], F32, tag="st", name=f"st{hp}_{pp}_1")
                nc.tensor.matmul(st0[:, 0, :], ko_lo, qT[0:E, s0],
                                 start=True, stop=True)
                nc.tensor.matmul(st0[:, 1, :], ko_hi, qT[E:P, s0],
                                 start=True, stop=True)
                nc.tensor.matmul(st1[:, 0, :], ko_lo, qT[0:E, s1],
                                 start=True, stop=True)
                nc.tensor.matmul(st1[:, 1, :], ko_hi, qT[E:P, s1],
                                 start=True, stop=True)
                if prev_pt is not None:
                    qq, qt0, qt1 = prev_pt
                    go = gob8[:, qq, :, :]
                    nc.tensor.matmul(agg0, go, qt0, start=False, stop=False,
                                     perf_mode=DR)
                    nc.tensor.matmul(agg1, go, qt1, start=False, stop=False,
                                     perf_mode=DR)
                pt0 = sb_pt.tile([P, 2, GW], FP8E5, tag="pt",
                                 name=f"pt{hp}_{pp}_0")
                pt1 = sb_pt.tile([P, 2, GW], FP8E5, tag="pt",
                                 name=f"pt{hp}_{pp}_1")
                # exp split: in hp0 DVE also carries the drip (casts/glu),
                # so ACT takes both groups every 8th pair; in hp1 strict
                # 1:1 (DVE carries the embedded epilogue instead).
                bonus = (pp % 8 == 3) if hp == 0 else False
                exp_unit(st0, pt0, "a")
                exp_unit(st1, pt1, "a" if bonus else "v")
                prev_pt = (pp, pt0, pt1)
                if hp == 0:
                    for fn in drip.get(pp, ()):
                        fn()
                else:
                    # hp0's ags columns are final: run its epilogue during
                    # hp1 (batched -- only two extra PSUM allocs total)
                    if pp == 6:
                        epi_half(0)
                    elif pp == 16:
                        epi_final(0, act_assist=True)
            qq, qt0, qt1 = prev_pt
            go = gob8[:, qq, :, :]
            nc.tensor.matmul(agg0, go, qt0, start=False, stop=True,
                             perf_mode=DR)
            nc.tensor.matmul(agg1, go, qt1, start=False, stop=True,
                             perf_mode=DR)
            agg_flush(g0, agg0, eng="v")
            agg_flush(g1, agg1, eng="a")

        # ---------------- epilogue tail (half 1) ----------
        epi_half(1)
        epi_final(1, act_assist=False)

        if dbg is not None:
            nc.sync.dma_start(dbg["qT"], qT)
            nc.sync.dma_start(dbg["gmT"], gmT)
            nc.sync.dma_start(dbg["ags"], ags)
            nc.sync.dma_start(dbg["okS"], okS)
            nc.sync.dma_start(dbg["gob8"],
                              gob8.rearrange("p a b c -> p (a b c)"))
            nc.sync.dma_start(dbg["out_pre"],
                              out_pre.rearrange("p a b -> p (a b)"))
            nc.sync.dma_start(dbg["mvC"], mvC.rearrange("p a b -> p (a b)"))


_CACHED = None


def _build(debug=False):
    global _CACHED
    if _CACHED is not None and not debug:
        return _CACHED
    nc = bacc.Bacc("TRN2", target_bir_lowering=False, debug=False)

    def din(name, shape, dt=F32):
        return nc.dram_tensor(name, shape, dt, kind="ExternalInput").ap()

    map_rows_d = din("map_rows", [NS, E])
    mapT_d = din("mapT", [E, NS], BF16)
    obsT_d = din("obsT", [E, NO], BF16)
    wpb_d = din("wpb", [E, BW], BF16)
    wpf_d = din("wpf", [E, FW], F32R)
    c8_d = din("c8", [P, MPAD - H], FP8)
    vec_d = din("vpack", [3 * E + 1])
    out_d = nc.dram_tensor("out", [NS, E], F32, kind="ExternalOutput").ap()

    dbg = None
    if debug:
        def dout(name, shape, dt=F32):
            return nc.dram_tensor(name, shape, dt, kind="ExternalOutput").ap()
        dbg = {
            "qT": dout("dbg_qT", [P, NS], BF16),
            "gmT": dout("dbg_gmT", [H + 1, NS], BF16),
            "ags": dout("dbg_ags", [H + 1, NS], F32R),
            "okS": dout("dbg_okS", [P, NO], BF16),
            "gob8": dout("dbg_gob8", [P, NPAIR * 2 * MPAD], FP8),
            "out_pre": dout("dbg_out_pre", [P, NT * E]),
            "mvC": dout("dbg_mvC", [P, NT * 2]),
        }

    with tile.TileContext(nc) as tc:
        _emit(tc, out_d, map_rows_d, mapT_d, obsT_d, wpb_d, wpf_d, c8_d,
              vec_d, dbg=dbg)
    nc.compile()
    if not debug:
        _CACHED = nc
    return nc


def _prep_in_maps(map_code, obs_code, Wq, Wk, Wv, Wo, bo, gamma, beta):
    f = np.float32
    map_code = np.ascontiguousarray(np.asarray(map_code, dtype=f))
    obs_code = np.asarray(obs_code, dtype=f)

    bf16_np = mybir.dt.np(BF16)
    fp8_np = mybir.dt.np(FP8)

    def to_bf16(x):
        return np.ascontiguousarray(np.asarray(x, dtype=f).astype(bf16_np))

    obsT = np.ascontiguousarray(obs_code.T)

    wq2 = np.concatenate([np.asarray(Wq, f), np.asarray(Wq, f)], axis=1)
    wk2 = np.concatenate([np.asarray(Wk, f), np.asarray(Wk, f)], axis=1)
    wpb = np.zeros((E, BW), dtype=f)
    wpb[:, _WQ0:_WQ0 + 128] = wq2
    wpb[:, _WK0:_WK0 + 128] = wk2
    wpb[:, _WV0:_WV0 + E] = np.asarray(Wv, f)
    wpb[:, _ONES0] = 1.0
    for k in range(H + 1):
        wpb[k, _IDO + k] = 1.0   # identity seed stationary [33, MPAD]

    woe = np.zeros((E, FW), dtype=f)
    woe[0:H, 0:E] = np.asarray(Wo, dtype=f)
    woe[H, E] = 1.0

    c8 = np.zeros((P, MPAD - H), dtype=fp8_np)
    c8[:, 0] = 1.0

    vpack = np.concatenate([
        np.asarray(bo, dtype=f), np.asarray(gamma, dtype=f),
        np.asarray(beta, dtype=f), np.full((1,), -SHIFT, dtype=f),
    ])
    shared = {
        "obsT": to_bf16(obsT),
        "wpb": to_bf16(wpb),
        "wpf": np.ascontiguousarray(woe),
        "c8": np.ascontiguousarray(c8),
        "vpack": np.ascontiguousarray(vpack),
    }
    in_maps = []
    for i in range(NCORES):
        shard = map_code[i * NS:(i + 1) * NS]
        m = dict(shared)
        m["map_rows"] = shard
        m["mapT"] = to_bf16(np.ascontiguousarray(shard.T))
        in_maps.append(m)
    return in_maps


def run(trace=False, **inputs):
    nc = _build()
    in_maps = _prep_in_maps(**inputs)
    res = run_bass_kernel_spmd(nc, in_maps, list(range(NCORES)), trace=trace)
    out = np.concatenate([res.results[i]["out"] for i in range(NCORES)], axis=0)
    return out, res


def kernel(**inputs):
    out, _ = run(trace=False, **inputs)
    return out


# revision 9
# speedup vs baseline: 1.1153x; 1.0362x over previous
"""Trainium2 Bass kernel for nn_Attention_5815385719367 (gnn_message_passing).

Computation (see reference):
  map_q/k/v = map_code @ Wq/Wk/Wv ; obs_k/v = obs_code @ Wk/Wv
  scores    = [sum(q*k,-1) | q @ obs_k.T] / 8
  w         = softmax(scores)
  agg       = w[:, :1]*glu(map_v) + w[:, 1:] @ glu(obs_v)
  out       = LN(agg @ Wo + bo + map_code) * gamma + beta

Sharding: data-parallel over N_map rows (2048 rows/core x 8 cores);
obs_code and weights replicated. No collectives.

v3 design notes (per core), building on v2:
  - scores computed TRANSPOSED in PSUM: ST[obs=128, map] via PE ROW-TILED
    pairs: even obs block's k.T sits on SBUF partitions 0-63 (PE tile
    (0,0)), odd block's on partitions 64-127 (tile (64,0)).
  - v3: qT/okS/gmT/ones/id33 are BF16 (v2 used f32r). f32 moving
    operands stream the PE at 2 cyc/col; bf16 streams 1 cyc/col, so the
    ST matmuls halve (427 -> ~220 ns each measured).
  - v3: okS keeps the k.T projection in its natural [128, NO] layout
    (dup halves from the wk|wk projection); the even/odd parity split
    is done by SLICING (partitions 0:64 x even cols / 64:128 x odd
    cols) -- v2's per-block parity copies were pure waste.
  - v3: a ~4us junk-matmul warmup burst runs during the input DMA so
    the PE HAM clock-gate opens (1.2 -> 2.4 GHz) before real work; v2
    ran the whole prologue + 10 pairs cold.
  - v3: lean prologue -- only map chunks 0,1 (hp0's groups), okS chunk
    pair 0 and gob batch 0 precede the main loop; map chunks 2,3 (only
    needed by hp1) and the rest of okS/gob drip into hp0.
  - softmax exp is the hard wall (1 elem/lane/cycle on ACT): split it
    between ACT (direct exp -> fp8 e5m2, logits shifted by +2) and DVE
    (Schraudolph: one mult-add tensor_scalar into uint8 whose bit
    pattern IS the e5m2 log-domain approximation).
  - PV runs fp8 DoubleRow over block PAIRS: stationary
    gob8[128, 2, 80] = glu(obs_v)|ones|zero-pad, moving pt8; the ones
    column accumulates the softmax denominator for free.
  - self-attention term folded into the PV accumulator seed via an
    identity-33 stationary matmul.
  - v3: tail epilogue (half 1) uses DVE for the normalize step and two
    big broadcast tensor_tensor ops for gamma/beta instead of 16
    small per-tile ops.
"""

import numpy as np

import concourse.bass as bass
import concourse.bacc as bacc
import concourse.tile as tile
from concourse import mybir
from concourse.bass_utils import run_bass_kernel_spmd

NCORES = 8
NM, NO, E = 16384, 8192, 64
NS = NM // NCORES            # 2048 map rows per core
H = E // 2                   # 32
TEMP = 8.0
EPS = 1e-6
P = 128
NT = NS // P                 # 16 row tiles per core
GW = 512                     # map group width (psum bank)
NPAIR = NO // 256            # 32 obs block-pairs
SHIFT = -2.0                 # logit shift: exp(l - SHIFT), cancels in ratio.
MPAD = 80                    # padded PV output partitions (33 real; %16 keeps
                             # the DoubleRow ldweights step legal, >64 keeps
                             # column tiling off)

F32 = mybir.dt.float32
F32R = mybir.dt.float32r
BF16 = mybir.dt.bfloat16
FP8 = mybir.dt.float8e4
FP8E5 = mybir.dt.float8e5
U8 = mybir.dt.uint8
AF = mybir.ActivationFunctionType
ALU = mybir.AluOpType
DR = mybir.MatmulPerfMode.DoubleRow

# Schraudolph constants for uint8 e5m2 log-domain exp of RAW score s:
#   i = 4*log2(exp(s/8 - SHIFT)) + 60 - sawtooth_center
SCH_A = 4.0 * 1.4426950408889634 / TEMP   # 0.72135
SCH_B = 60.0 - 4.0 * 1.4426950408889634 * SHIFT - 0.229

# layout of the bf16 weight pack [64, BW]
_WQ0 = 0              # wq duplicated [64, 128]
_WK0 = 128            # wk duplicated [64, 128]
_WV0 = 256            # wv [64, 64]
_ONES0 = 320          # ones column [64, 1]
_IDO = 321            # identity [33, MPAD] seed stationary
BW = 321 + MPAD

# layout of the f32r pack [64, FW]: woe only
FW = E + 2


def _bc_part(ap, n):
    """Broadcast a [x, ...] AP along a new leading partition dim of n."""
    return bass.AP(tensor=ap.tensor, offset=ap.offset, ap=[[0, n]] + list(ap.ap))


def _emit(tc, out_d, map_rows_d, mapT_d, obsT_d, wpb_d, wpf_d, vec_d,
          dbg=None):
    nc = tc.nc
    with tc.tile_pool(name="consts", bufs=1) as consts, \
         tc.tile_pool(name="big", bufs=1) as big, \
         tc.tile_pool(name="sb_sm", bufs=3) as sb_sm, \
         tc.tile_pool(name="sb_pt", bufs=4) as sb_pt, \
         tc.tile_pool(name="ps", bufs=3, space="PSUM") as ps, \
         tc.tile_pool(name="ps_agg", bufs=2, space="PSUM") as ps_agg:

        # ---------------- constants ----------------
        wpb = consts.tile([E, BW], BF16)          # bf16 weights pack
        nc.sync.dma_start(wpb, wpb_d)
        wq2 = wpb[:, _WQ0:_WQ0 + 128]             # [64,128] wq|wq
        wk2 = wpb[:, _WK0:_WK0 + 128]             # [64,128] wk|wk
        wv = wpb[:, _WV0:_WV0 + E]                # [64,64]
        ones64 = wpb[:, _ONES0:_ONES0 + 1]
        id33 = wpb[0:H + 1, _IDO:_IDO + MPAD]     # identity seed [33, 80]

        vecs = consts.tile([P, 3 * E + 1], F32)   # bo|gamma|beta|-shift
        nc.sync.dma_start(vecs, _bc_part(vec_d, P))
        bo_b = vecs[:, 0:E]
        ga_b = vecs[:, E:2 * E]
        be_b = vecs[:, 2 * E:3 * E]
        msh = vecs[:, 3 * E:3 * E + 1]            # -SHIFT bias column

        wpf = consts.tile([E, FW], F32R)
        nc.sync.dma_start(wpf, wpf_d)
        woe = wpf[0:H + 1, 0:E + 2]               # [33,66]

        # ---------------- warmup: open the HAM clock gate --------------
        # ~22 junk matmuls (~6us cold = >1 full HAM window) on the weight
        # pack while the input DMAs land; output PSUM is never read.
        for wi in range(22):
            wu = ps.tile([P, 2, GW], F32, tag="st", name=f"wu{wi}")
            nc.tensor.matmul(wu[:, 0, 0:320], wq2, wpb[:, 0:320],
                             start=True, stop=True)

        # ---------------- big arenas + input DMAs ----------------
        # NOTE: every dma_start dispatch costs ~600ns SERIAL time on the
        # issuing engine's queue -- keep the count low and the order
        # matched to consumption so the sxp_fold bounces (enqueued after
        # these) dispatch early.
        mapT = big.tile([E, NS], BF16)
        obsT = big.tile([E, NO], BF16)
        for lo, hi, t_, s_ in ((0, 512, mapT, mapT_d),
                               (0, 1024, obsT, obsT_d),
                               (512, 1024, mapT, mapT_d),
                               (1024, 2048, obsT, obsT_d),
                               (1024, 2048, mapT, mapT_d),
                               (2048, 8192, obsT, obsT_d)):
            nc.sync.dma_start(t_[:, lo:hi], s_[:, lo:hi])
        map_rows = big.tile([P, NT, E], F32)
        nc.scalar.dma_start(map_rows, map_rows_d.rearrange("(t p) e -> p t e", p=P))

        qT = big.tile([P, NS], BF16)              # map_q.T duplicated halves
        gmT = big.tile([H + 1, NS], BF16)         # [glu(map_v).T ; selfexp]
        okS = big.tile([P, NO], BF16)             # obs_k.T duplicated halves
        gob8 = big.tile([P, NPAIR, 2, MPAD], FP8)  # glu(obs_v)|1|0 pairs
        ags = big.tile([H + 1, NS], F32R)         # [numer.T ; denom]
        map_pb = big.tile([P, NT, E], F32)        # map + bo
        out_pre = big.tile([P, NT, E], F32)
        out_all = big.tile([P, NT, E], F32)
        mvC = big.tile([P, NT, 2], F32)
        rstd = big.tile([P, NT], F32)

        # gob8 static columns: ones at h=32, zeros at h=33..79. memset on
        # the (idle) gpsimd -- the v2/v3 broadcast-DMA version cost ~10us
        # of serial dispatch on the sync queue.
        gob8f = gob8.rearrange("p a b c -> p (a b) c")
        nc.gpsimd.memset(gob8f[:, :, H:H + 1], 1.0)
        nc.gpsimd.memset(gob8f[:, :, H + 1:MPAD], 0.0)

        # ---------------- prologue pieces ----------------
        # self-exp fold helper state: bounce selfexp through HBM for a
        # partition-broadcast read (engines cannot partition-broadcast).
        sxp_hbm = nc.dram_tensor("sxp_hbm", [NS], BF16, kind="Internal").ap()
        sxp = big.tile([H, NS], BF16)

        def map_chunk_a(c):
            """map_q (duplicated) for map cols [c*512, ..)."""
            sl = slice(c * GW, (c + 1) * GW)
            q_ps = ps.tile([P, 2, GW], F32, tag="st", name=f"qps{c}")
            nc.tensor.matmul(q_ps[:, 0, :], wq2, mapT[:, sl],
                             start=True, stop=True)
            nc.vector.tensor_copy(qT[:, sl], q_ps[:, 0, :])

        def map_chunk_b(c):
            """map_k, self-score, selfexp for map cols [c*512, ..)."""
            sl = slice(c * GW, (c + 1) * GW)
            k_ps = ps.tile([P, 2, GW], F32, tag="st", name=f"kps{c}")
            nc.tensor.matmul(k_ps[:, 0, :], wk2, mapT[:, sl],
                             start=True, stop=True)
            qk = sb_sm.tile([E, GW], BF16, tag="qk", name=f"qk{c}")
            nc.vector.tensor_tensor(out=qk, in0=qT[0:E, sl],
                                    in1=k_ps[0:E, 0, :], op=ALU.mult)
            # self-score sum lands in the unused upper half of k_ps
            ss_ps = k_ps[0:1, 1, :]
            nc.tensor.matmul(ss_ps, ones64, qk, start=True, stop=True)
            nc.scalar.activation(gmT[H:H + 1, sl], ss_ps, AF.Exp,
                                 scale=1.0 / TEMP, bias=msh[0:1])

        def map_chunk_c(c):
            """glu(map_v) for map cols [c*512, ..)."""
            sl = slice(c * GW, (c + 1) * GW)
            v_ps = ps.tile([P, 2, GW], F32, tag="st", name=f"vps{c}")
            nc.tensor.matmul(v_ps[0:E, 0, :], wv, mapT[:, sl],
                             start=True, stop=True)
            th = sb_sm.tile([H, GW], F32, tag="th", name=f"th{c}")
            nc.scalar.activation(th, v_ps[H:E, 0, :], AF.Tanh, scale=0.5)
            nc.gpsimd.tensor_scalar(out=th, in0=th, scalar1=0.5, scalar2=0.5,
                                    op0=ALU.mult, op1=ALU.add)
            nc.vector.tensor_tensor(out=gmT[0:H, sl], in0=v_ps[0:H, 0, :],
                                    in1=th, op=ALU.mult)

        def sxp_fold(c):
            sl = slice(c * GW, (c + 1) * GW)
            nc.sync.dma_start(sxp_hbm[sl], gmT[H:H + 1, sl])
            nc.sync.dma_start(sxp[:, sl], _bc_part(sxp_hbm[sl], H))
            nc.vector.tensor_tensor(out=gmT[0:H, sl], in0=gmT[0:H, sl],
                                    in1=sxp[:, sl], op=ALU.mult)

        def okS_chunk2(c2, eng="v"):
            """obs_k.T (dup halves) for obs cols [c2*1024, ..): 2 matmuls
            into one PSUM tile, ONE bulk cast out. No parity shuffling --
            the ST matmuls slice [0:64] x even cols / [64:128] x odd."""
            k_ps = ps.tile([P, 2, GW], F32, tag="st", name=f"okps{c2}")
            for t in range(2):
                c = 2 * c2 + t
                sl = slice(c * GW, (c + 1) * GW)
                nc.tensor.matmul(k_ps[:, t, :], wk2, obsT[:, sl],
                                 start=True, stop=True)
            dst = okS[:, c2 * 2 * GW:(c2 + 1) * 2 * GW]
            src = k_ps.rearrange("p a b -> p (a b)")
            if eng == "v":
                nc.vector.tensor_copy(dst, src)
            else:
                nc.scalar.copy(dst, src)

        def obs_v_batch2(c2, nb=16):
            """glu(obs_v) for nb consecutive obs blocks (one PSUM alloc)."""
            v_ps = ps.tile([P, 16, E], F32, tag="st", name=f"ovps{c2}")
            for b in range(nb):
                blk = c2 * 16 + b
                nc.tensor.matmul(v_ps[:, b, :],
                                 obsT[:, blk * P:(blk + 1) * P], wv,
                                 start=True, stop=True)
            tho = sb_sm.tile([P, 16, H], F32, tag="tho", name=f"tho{c2}")
            nc.scalar.activation(tho[:, 0:nb, :], v_ps[:, 0:nb, H:E],
                                 AF.Tanh, scale=0.5)
            nc.gpsimd.tensor_scalar(out=tho[:, 0:nb, :], in0=tho[:, 0:nb, :],
                                    scalar1=0.5, scalar2=0.5,
                                    op0=ALU.mult, op1=ALU.add)
            # blocks 16*c2.. -> pairs 8*c2.., t = parity
            og = gob8[:, 8 * c2:8 * c2 + nb // 2, :, 0:H]
            vi = v_ps[:, 0:nb, 0:H].rearrange("p (a b) h -> p a b h", b=2)
            ti = tho[:, 0:nb, :].rearrange("p (a b) h -> p a b h", b=2)
            nc.vector.tensor_tensor(out=og, in0=vi, in1=ti, op=ALU.mult)

        def map_pb_fill():
            bo_rep = bass.AP(tensor=bo_b.tensor, offset=bo_b.offset,
                             ap=[list(bo_b.ap[0]), [0, NT], [1, E]])
            nc.gpsimd.tensor_tensor(out=map_pb, in0=map_rows, in1=bo_rep,
                                    op=ALU.add)

        def agg_flush(g, agg, eng="v"):
            sl = slice(g * GW, (g + 1) * GW)
            if eng == "v":
                nc.vector.tensor_copy(ags[0:H + 1, sl], agg[0:H + 1, :])
            else:
                nc.scalar.copy(ags[0:H + 1, sl], agg[0:H + 1, :])

        # ---------------- epilogue ----------------
        def epi_half(half):
            """Batched epilogue for 8 map tiles: all PE matmuls first,
            then the elementwise chain pipelines on DVE."""
            base = half * (NT // 2)
            uda = ps.tile([P, 8, P], F32, tag="st", name=f"uda{half}")
            for i in range(8):
                sl = slice((base + i) * P, (base + i + 1) * P)
                nc.tensor.matmul(uda[:, i, 0:E + 2], ags[:, sl], woe,
                                 start=True, stop=True)
            uds = sb_sm.tile([P, 8, E + 2], F32, tag="uds", name=f"uds{half}")
            nc.vector.tensor_copy(uds, uda[:, :, 0:E + 2])
            rden = sb_sm.tile([P, 8], F32, tag="rden", name=f"rden{half}")
            nc.vector.reciprocal(rden, uds[:, :, E])
            for i in range(8):
                t = base + i
                nc.vector.scalar_tensor_tensor(out=out_pre[:, t, :],
                                               in0=uds[:, i, 0:E],
                                               scalar=rden[:, i:i + 1],
                                               in1=map_pb[:, t, :],
                                               op0=ALU.mult, op1=ALU.add)
                stats = sb_sm.tile([P, 6], F32, tag="stats", name=f"stats{t}")
                nc.vector.bn_stats(stats, out_pre[:, t, :])
                nc.vector.bn_aggr(mvC[:, t, :], stats)

        def epi_rstd(half):
            """1/sqrt(var+eps) for 8 tiles via min-poly + 2 NR steps."""
            tsl = slice(half * (NT // 2), (half + 1) * (NT // 2))
            w = NT // 2
            vpe = sb_sm.tile([P, w], F32, tag="vpe", name=f"vpe{half}")
            nc.vector.tensor_scalar_add(vpe, mvC[:, tsl, 1], EPS)
            c1 = sb_sm.tile([P, w], F32, tag="nc1", name=f"nc1{half}")
            nc.vector.tensor_scalar(out=c1, in0=vpe, scalar1=0.564185,
                                    scalar2=0.378467, op0=ALU.mult,
                                    op1=ALU.add)
            c2 = sb_sm.tile([P, w], F32, tag="nc2", name=f"nc2{half}")
            nc.vector.tensor_scalar(out=c2, in0=vpe, scalar1=0.288949,
                                    scalar2=0.791321, op0=ALU.mult,
                                    op1=ALU.add)
            nc.vector.tensor_tensor(out=c1, in0=c1, in1=c2, op=ALU.min)
            rs = rstd[:, tsl]
            nc.vector.reciprocal(rs, c1)
            for _ in range(2):
                nc.vector.tensor_tensor(out=c1, in0=rs, in1=rs, op=ALU.mult)
                nc.vector.tensor_tensor(out=c1, in0=c1, in1=vpe, op=ALU.mult)
                nc.vector.tensor_scalar(out=c1, in0=c1, scalar1=-0.5,
                                        scalar2=1.5, op0=ALU.mult,
                                        op1=ALU.add)
                nc.vector.tensor_tensor(out=rs, in0=rs, in1=c1, op=ALU.mult)

        def epi_norm(half, act_assist=True):
            """Normalize + gamma/beta + output DMA for one half (rstd must
            already be computed via epi_rstd)."""
            t0, t1 = half * (NT // 2), (half + 1) * (NT // 2)
            for t in range(t0, t1):
                # tail: split tiles between ACT (identity w/ bias+scale)
                # and DVE; embedded: all on ACT to keep DVE free
                on_act = act_assist or (t - t0) < 4
                if on_act:
                    nmr = sb_sm.tile([P, 1], F32, tag="nmr", name=f"nmr{t}")
                    nc.vector.tensor_scalar(out=nmr, in0=mvC[:, t, 0:1],
                                            scalar1=rstd[:, t:t + 1],
                                            scalar2=-1.0, op0=ALU.mult,
                                            op1=ALU.mult)
                    if act_assist:
                        xn = sb_sm.tile([P, E], F32, tag="xn", name=f"xn{t}")
                        nc.scalar.activation(xn, out_pre[:, t, :],
                                             AF.Identity, bias=nmr,
                                             scale=rstd[:, t:t + 1])
                        nc.gpsimd.tensor_tensor(out=xn, in0=xn, in1=ga_b,
                                                op=ALU.mult)
                        nc.gpsimd.tensor_tensor(out=out_all[:, t, :], in0=xn,
                                                in1=be_b, op=ALU.add)
                    else:
                        nc.scalar.activation(out_all[:, t, :],
                                             out_pre[:, t, :],
                                             AF.Identity, bias=nmr,
                                             scale=rstd[:, t:t + 1])
                else:
                    nc.vector.tensor_scalar(out=out_all[:, t, :],
                                            in0=out_pre[:, t, :],
                                            scalar1=mvC[:, t, 0:1],
                                            scalar2=rstd[:, t:t + 1],
                                            op0=ALU.subtract, op1=ALU.mult)
            if not act_assist:
                # two big broadcast ops over all 8 tiles at once
                ga_rep = bass.AP(tensor=ga_b.tensor, offset=ga_b.offset,
                                 ap=[list(ga_b.ap[0]), [0, t1 - t0], [1, E]])
                be_rep = bass.AP(tensor=be_b.tensor, offset=be_b.offset,
                                 ap=[list(be_b.ap[0]), [0, t1 - t0], [1, E]])
                nc.vector.tensor_tensor(out=out_all[:, t0:t1, :],
                                        in0=out_all[:, t0:t1, :],
                                        in1=ga_rep, op=ALU.mult)
                nc.vector.tensor_tensor(out=out_all[:, t0:t1, :],
                                        in0=out_all[:, t0:t1, :],
                                        in1=be_rep, op=ALU.add)
            od = out_d.rearrange("(t p) e -> p t e", p=P)
            for q in range(2):
                qsl = slice(t0 + q * (NT // 4), t0 + (q + 1) * (NT // 4))
                nc.sync.dma_start(od[:, qsl, :], out_all[:, qsl, :])

        # ---------------- prologue head (lean) ----------------
        # Only what hp0's first pairs need: map chunks 0,1 (group cols),
        # okS chunks 0,1 (pairs 0-3), gob batch 0 (pairs 0-7).
        map_chunk_a(0)
        map_chunk_b(0)
        map_chunk_c(0)
        sxp_fold(0)
        okS_chunk2(0)
        map_chunk_a(1)
        map_chunk_b(1)
        map_chunk_c(1)
        sxp_fold(1)
        obs_v_batch2(0)

        # drip schedule for hp0: item lists per pair index.
        # okS chunk c2 must land before pair 4*c2; gob batch b before
        # pair 8*b; map chunks 2,3 (qT/gmT for hp1) anywhere before hp1.
        drip = {
            0: [lambda: okS_chunk2(1)],
            2: [lambda: okS_chunk2(2)],
            4: [lambda: obs_v_batch2(1)],
            6: [lambda: okS_chunk2(3)],
            8: [lambda: okS_chunk2(4)],
            10: [lambda: obs_v_batch2(2)],
            12: [lambda: okS_chunk2(5), lambda: map_pb_fill()],
            14: [lambda: okS_chunk2(6)],
            16: [lambda: obs_v_batch2(3)],
            18: [lambda: okS_chunk2(7)],
            20: [lambda: map_chunk_a(2)],
            21: [lambda: map_chunk_b(2)],
            22: [lambda: map_chunk_c(2)],
            23: [lambda: sxp_fold(2)],
            24: [lambda: map_chunk_a(3)],
            25: [lambda: map_chunk_b(3)],
            26: [lambda: map_chunk_c(3)],
            27: [lambda: sxp_fold(3)],
        }

        def exp_unit(st_t, pt_t, eng):
            if eng == "a":
                nc.scalar.activation(pt_t, st_t, AF.Exp,
                                     scale=1.0 / TEMP, bias=msh)
            else:
                nc.vector.tensor_scalar(out=pt_t.bitcast(U8), in0=st_t,
                                        scalar1=SCH_A, scalar2=SCH_B,
                                        op0=ALU.mult, op1=ALU.add)

        # ---------------- main loop: 2 half-passes x 32 pairs ----------
        # Software-pipelined by one pair: the PV for pair p-1 is issued to
        # the PE AFTER pair p's ST matmuls, so by the time the PE FIFO
        # reaches it, exp(p-1) has long finished.
        for hp in range(2):
            agg0 = ps_agg.tile([MPAD, GW], F32, tag="agg", name=f"agg{hp}_0")
            agg1 = ps_agg.tile([MPAD, GW], F32, tag="agg", name=f"agg{hp}_1")
            g0 = 2 * hp
            g1 = 2 * hp + 1
            s0 = slice(g0 * GW, (g0 + 1) * GW)
            s1 = slice(g1 * GW, (g1 + 1) * GW)
            nc.tensor.matmul(agg0, id33, gmT[:, s0],
                             start=True, stop=False)
            nc.tensor.matmul(agg1, id33, gmT[:, s1],
                             start=True, stop=False)
            prev_pt = None
            for pp in range(NPAIR):
                co = 256 * pp
                ko_lo = okS[0:E, co:co + P]
                ko_hi = okS[E:P, co + P:co + 2 * P]
                st0 = ps.tile([P, 2, GW], F32, tag="st", name=f"st{hp}_{pp}_0")
                st1 = ps.tile([P, 2, GW], F32, tag="st", name=f"st{hp}_{pp}_1")
                nc.tensor.matmul(st0[:, 0, :], ko_lo, qT[0:E, s0],
                                 start=True, stop=True)
                nc.tensor.matmul(st0[:, 1, :], ko_hi, qT[E:P, s0],
                                 start=True, stop=True)
                nc.tensor.matmul(st1[:, 0, :], ko_lo, qT[0:E, s1],
                                 start=True, stop=True)
                nc.tensor.matmul(st1[:, 1, :], ko_hi, qT[E:P, s1],
                                 start=True, stop=True)
                if prev_pt is not None:
                    qq, qt0, qt1 = prev_pt
                    go = gob8[:, qq, :, :]
                    nc.tensor.matmul(agg0, go, qt0, start=False, stop=False,
                                     perf_mode=DR)
                    nc.tensor.matmul(agg1, go, qt1, start=False, stop=False,
                                     perf_mode=DR)
                pt0 = sb_pt.tile([P, 2, GW], FP8E5, tag="pt",
                                 name=f"pt{hp}_{pp}_0")
                pt1 = sb_pt.tile([P, 2, GW], FP8E5, tag="pt",
                                 name=f"pt{hp}_{pp}_1")
                # exp split: mostly 1:1 ACT/DVE with a few ACT "bonus"
                # pairs to offset DVE's drip (hp0) / epilogue (hp1) load.
                bonus = pp in ((10, 21) if hp == 0 else (6, 19))
                exp_unit(st0, pt0, "a")
                exp_unit(st1, pt1, "a" if bonus else "v")
                prev_pt = (pp, pt0, pt1)
                if hp == 0:
                    for fn in drip.get(pp, ()):
                        fn()
                else:
                    # hp0's ags columns are final: run its epilogue during
                    # hp1 (batched -- only two extra PSUM allocs total)
                    if pp == 6:
                        epi_half(0)
                    elif pp == 9:
                        epi_rstd(0)
                    elif pp == 16:
                        epi_norm(0, act_assist=True)
            qq, qt0, qt1 = prev_pt
            go = gob8[:, qq, :, :]
            nc.tensor.matmul(agg0, go, qt0, start=False, stop=True,
                             perf_mode=DR)
            nc.tensor.matmul(agg1, go, qt1, start=False, stop=True,
                             perf_mode=DR)
            agg_flush(g0, agg0, eng="v")
            agg_flush(g1, agg1, eng="a")

        # ---------------- epilogue tail (half 1) ----------
        epi_half(1)
        epi_rstd(1)
        epi_norm(1, act_assist=False)

        if dbg is not None:
            nc.sync.dma_start(dbg["qT"], qT)
            nc.sync.dma_start(dbg["gmT"], gmT)
            nc.sync.dma_start(dbg["ags"], ags)
            nc.sync.dma_start(dbg["okS"], okS)
            nc.sync.dma_start(dbg["gob8"],
                              gob8.rearrange("p a b c -> p (a b c)"))
            nc.sync.dma_start(dbg["out_pre"],
                              out_pre.rearrange("p a b -> p (a b)"))
            nc.sync.dma_start(dbg["mvC"], mvC.rearrange("p a b -> p (a b)"))


_CACHED = None


def _build(debug=False):
    global _CACHED
    if _CACHED is not None and not debug:
        return _CACHED
    nc = bacc.Bacc("TRN2", target_bir_lowering=False, debug=False)

    def din(name, shape, dt=F32):
        return nc.dram_tensor(name, shape, dt, kind="ExternalInput").ap()

    map_rows_d = din("map_rows", [NS, E])
    mapT_d = din("mapT", [E, NS], BF16)
    obsT_d = din("obsT", [E, NO], BF16)
    wpb_d = din("wpb", [E, BW], BF16)
    wpf_d = din("wpf", [E, FW], F32R)
    vec_d = din("vpack", [3 * E + 1])
    out_d = nc.dram_tensor("out", [NS, E], F32, kind="ExternalOutput").ap()

    dbg = None
    if debug:
        def dout(name, shape, dt=F32):
            return nc.dram_tensor(name, shape, dt, kind="ExternalOutput").ap()
        dbg = {
            "qT": dout("dbg_qT", [P, NS], BF16),
            "gmT": dout("dbg_gmT", [H + 1, NS], BF16),
            "ags": dout("dbg_ags", [H + 1, NS], F32R),
            "okS": dout("dbg_okS", [P, NO], BF16),
            "gob8": dout("dbg_gob8", [P, NPAIR * 2 * MPAD], FP8),
            "out_pre": dout("dbg_out_pre", [P, NT * E]),
            "mvC": dout("dbg_mvC", [P, NT * 2]),
        }

    with tile.TileContext(nc) as tc:
        _emit(tc, out_d, map_rows_d, mapT_d, obsT_d, wpb_d, wpf_d,
              vec_d, dbg=dbg)
    nc.compile()
    if not debug:
        _CACHED = nc
    return nc


def _prep_in_maps(map_code, obs_code, Wq, Wk, Wv, Wo, bo, gamma, beta):
    f = np.float32
    map_code = np.ascontiguousarray(np.asarray(map_code, dtype=f))
    obs_code = np.asarray(obs_code, dtype=f)

    bf16_np = mybir.dt.np(BF16)

    def to_bf16(x):
        return np.ascontiguousarray(np.asarray(x, dtype=f).astype(bf16_np))

    obsT = np.ascontiguousarray(obs_code.T)

    wq2 = np.concatenate([np.asarray(Wq, f), np.asarray(Wq, f)], axis=1)
    wk2 = np.concatenate([np.asarray(Wk, f), np.asarray(Wk, f)], axis=1)
    wpb = np.zeros((E, BW), dtype=f)
    wpb[:, _WQ0:_WQ0 + 128] = wq2
    wpb[:, _WK0:_WK0 + 128] = wk2
    wpb[:, _WV0:_WV0 + E] = np.asarray(Wv, f)
    wpb[:, _ONES0] = 1.0
    for k in range(H + 1):
        wpb[k, _IDO + k] = 1.0   # identity seed stationary [33, MPAD]

    woe = np.zeros((E, FW), dtype=f)
    woe[0:H, 0:E] = np.asarray(Wo, dtype=f)
    woe[H, E] = 1.0

    vpack = np.concatenate([
        np.asarray(bo, dtype=f), np.asarray(gamma, dtype=f),
        np.asarray(beta, dtype=f), np.full((1,), -SHIFT, dtype=f),
    ])
    shared = {
        "obsT": to_bf16(obsT),
        "wpb": to_bf16(wpb),
        "wpf": np.ascontiguousarray(woe),
        "vpack": np.ascontiguousarray(vpack),
    }
    in_maps = []
    for i in range(NCORES):
        shard = map_code[i * NS:(i + 1) * NS]
        m = dict(shared)
        m["map_rows"] = shard
        m["mapT"] = to_bf16(np.ascontiguousarray(shard.T))
        in_maps.append(m)
    return in_maps


def run(trace=False, **inputs):
    nc = _build()
    in_maps = _prep_in_maps(**inputs)
    res = run_bass_kernel_spmd(nc, in_maps, list(range(NCORES)), trace=trace)
    out = np.concatenate([res.results[i]["out"] for i in range(NCORES)], axis=0)
    return out, res


def kernel(**inputs):
    out, _ = run(trace=False, **inputs)
    return out


# revision 15
# speedup vs baseline: 1.1442x; 1.0259x over previous
"""Trainium2 Bass kernel for nn_Attention_5815385719367 (gnn_message_passing).

Computation (see reference):
  map_q/k/v = map_code @ Wq/Wk/Wv ; obs_k/v = obs_code @ Wk/Wv
  scores    = [sum(q*k,-1) | q @ obs_k.T] / 8
  w         = softmax(scores)
  agg       = w[:, :1]*glu(map_v) + w[:, 1:] @ glu(obs_v)
  out       = LN(agg @ Wo + bo + map_code) * gamma + beta

Sharding: data-parallel over N_map rows (2048 rows/core x 8 cores);
obs_code and weights replicated. No collectives.

v3 design notes (per core), building on v2:
  - scores computed TRANSPOSED in PSUM: ST[obs=128, map] via PE ROW-TILED
    pairs: even obs block's k.T sits on SBUF partitions 0-63 (PE tile
    (0,0)), odd block's on partitions 64-127 (tile (64,0)).
  - v3: qT/okS/gmT/ones/id33 are BF16 (v2 used f32r). f32 moving
    operands stream the PE at 2 cyc/col; bf16 streams 1 cyc/col, so the
    ST matmuls halve (427 -> ~220 ns each measured).
  - v3: okS keeps the k.T projection in its natural [128, NO] layout
    (dup halves from the wk|wk projection); the even/odd parity split
    is done by SLICING (partitions 0:64 x even cols / 64:128 x odd
    cols) -- v2's per-block parity copies were pure waste.
  - v3: a ~4us junk-matmul warmup burst runs during the input DMA so
    the PE HAM clock-gate opens (1.2 -> 2.4 GHz) before real work; v2
    ran the whole prologue + 10 pairs cold.
  - v3: lean prologue -- only map chunks 0,1 (hp0's groups), okS chunk
    pair 0 and gob batch 0 precede the main loop; map chunks 2,3 (only
    needed by hp1) and the rest of okS/gob drip into hp0.
  - softmax exp is the hard wall (1 elem/lane/cycle on ACT): split it
    between ACT (direct exp -> fp8 e5m2, logits shifted by +2) and DVE
    (Schraudolph: one mult-add tensor_scalar into uint8 whose bit
    pattern IS the e5m2 log-domain approximation).
  - PV runs fp8 DoubleRow over block PAIRS: stationary
    gob8[128, 2, 80] = glu(obs_v)|ones|zero-pad, moving pt8; the ones
    column accumulates the softmax denominator for free.
  - self-attention term folded into the PV accumulator seed via an
    identity-33 stationary matmul.
  - v3: tail epilogue (half 1) uses DVE for the normalize step and two
    big broadcast tensor_tensor ops for gamma/beta instead of 16
    small per-tile ops.
"""

import numpy as np

import concourse.bass as bass
import concourse.bacc as bacc
import concourse.tile as tile
from concourse import mybir
from concourse.bass_utils import run_bass_kernel_spmd

NCORES = 8
NM, NO, E = 16384, 8192, 64
NS = NM // NCORES            # 2048 map rows per core
H = E // 2                   # 32
TEMP = 8.0
EPS = 1e-6
P = 128
NT = NS // P                 # 16 row tiles per core
GW = 512                     # map group width (psum bank)
NPAIR = NO // 256            # 32 obs block-pairs
SHIFT = -2.0                 # logit shift: exp(l - SHIFT), cancels in ratio.
MPAD = 80                    # padded PV output partitions (33 real; %16 keeps
                             # the DoubleRow ldweights step legal, >64 keeps
                             # column tiling off)

F32 = mybir.dt.float32
F32R = mybir.dt.float32r
BF16 = mybir.dt.bfloat16
FP8 = mybir.dt.float8e4
FP8E5 = mybir.dt.float8e5
U8 = mybir.dt.uint8
AF = mybir.ActivationFunctionType
ALU = mybir.AluOpType
DR = mybir.MatmulPerfMode.DoubleRow

# Schraudolph constants for uint8 e5m2 log-domain exp of RAW score s:
#   i = 4*log2(exp(s/8 - SHIFT)) + 60 - sawtooth_center
SCH_A = 4.0 * 1.4426950408889634 / TEMP   # 0.72135
SCH_B = 60.0 - 4.0 * 1.4426950408889634 * SHIFT - 0.229

# layout of the bf16 weight pack [64, BW]
_WQ0 = 0              # wq duplicated [64, 128]
_WK0 = 128            # wk duplicated [64, 128]
_WV0 = 256            # wv [64, 64]
_ONES0 = 320          # ones column [64, 1]
_IDO = 321            # identity [33, MPAD] seed stationary
BW = 321 + MPAD

# layout of the f32r pack [64, FW]: woe only
FW = E + 2


def _bc_part(ap, n):
    """Broadcast a [x, ...] AP along a new leading partition dim of n."""
    return bass.AP(tensor=ap.tensor, offset=ap.offset, ap=[[0, n]] + list(ap.ap))


def _emit(tc, out_d, map_rows_d, mapT_d, obsT_d, wpb_d, wpf_d, vec_d,
          dbg=None):
    nc = tc.nc
    with tc.tile_pool(name="consts", bufs=1) as consts, \
         tc.tile_pool(name="big", bufs=1) as big, \
         tc.tile_pool(name="sb_sm", bufs=3) as sb_sm, \
         tc.tile_pool(name="sb_pt", bufs=6) as sb_pt, \
         tc.tile_pool(name="ps", bufs=3, space="PSUM") as ps, \
         tc.tile_pool(name="ps_agg", bufs=2, space="PSUM") as ps_agg:

        # ---------------- constants ----------------
        wpb = consts.tile([E, BW], BF16)          # bf16 weights pack
        nc.sync.dma_start(wpb, wpb_d)
        wq2 = wpb[:, _WQ0:_WQ0 + 128]             # [64,128] wq|wq
        wk2 = wpb[:, _WK0:_WK0 + 128]             # [64,128] wk|wk
        wv = wpb[:, _WV0:_WV0 + E]                # [64,64]
        ones64 = wpb[:, _ONES0:_ONES0 + 1]
        id33 = wpb[0:H + 1, _IDO:_IDO + MPAD]     # identity seed [33, 80]

        vecs = consts.tile([P, 3 * E + 1], F32)   # bo|gamma|beta|-shift
        nc.sync.dma_start(vecs, _bc_part(vec_d, P))
        bo_b = vecs[:, 0:E]
        ga_b = vecs[:, E:2 * E]
        be_b = vecs[:, 2 * E:3 * E]
        msh = vecs[:, 3 * E:3 * E + 1]            # -SHIFT bias column

        wpf = consts.tile([E, FW], F32R)
        nc.sync.dma_start(wpf, wpf_d)
        woe = wpf[0:H + 1, 0:E + 2]               # [33,66]

        # ---------------- warmup: open the HAM clock gate --------------
        # ~36 junk matmuls (~9.6us cold -- the observed throttle windows
        # are ~6.8us, so cover two full windows) on the weight pack while
        # the input DMAs land; output PSUM is never read.
        for wi in range(36):
            wu = ps.tile([P, 2, GW], F32, tag="st", name=f"wu{wi}")
            nc.tensor.matmul(wu[:, 0, 0:320], wq2, wpb[:, 0:320],
                             start=True, stop=True)

        # ---------------- big arenas + input DMAs ----------------
        # NOTE: every dma_start dispatch costs ~600ns SERIAL time on the
        # issuing engine's queue -- keep the count low and the order
        # matched to consumption so the sxp_fold bounces (enqueued after
        # these) dispatch early.
        mapT = big.tile([E, NS], BF16)
        obsT = big.tile([E, NO], BF16)
        for lo, hi, t_, s_ in ((0, 512, mapT, mapT_d),
                               (0, 1024, obsT, obsT_d),
                               (512, 1024, mapT, mapT_d),
                               (1024, 2048, obsT, obsT_d),
                               (1024, 2048, mapT, mapT_d),
                               (2048, 8192, obsT, obsT_d)):
            nc.sync.dma_start(t_[:, lo:hi], s_[:, lo:hi])
        map_rows = big.tile([P, NT, E], F32)
        nc.scalar.dma_start(map_rows, map_rows_d.rearrange("(t p) e -> p t e", p=P))

        qT = big.tile([P, NS], BF16)              # map_q.T duplicated halves
        gmT = big.tile([H + 1, NS], BF16)         # [glu(map_v).T ; selfexp]
        okS = big.tile([P, NO], BF16)             # obs_k.T duplicated halves
        gob8 = big.tile([P, NPAIR, 2, MPAD], FP8)  # glu(obs_v)|1|0 pairs
        ags = big.tile([H + 1, NS], F32R)         # [numer.T ; denom]
        map_pb = big.tile([P, NT, E], F32)        # map + bo
        out_pre = big.tile([P, NT, E], F32)
        out_all = big.tile([P, NT, E], F32)
        mvC = big.tile([P, NT, 2], F32)
        rstd = big.tile([P, NT], F32)

        # gob8 static columns: ones at h=32, zeros at h=33..79. memset on
        # the (idle) gpsimd -- the v2/v3 broadcast-DMA version cost ~10us
        # of serial dispatch on the sync queue.
        gob8f = gob8.rearrange("p a b c -> p (a b) c")
        nc.gpsimd.memset(gob8f[:, :, H:H + 1], 1.0)
        nc.gpsimd.memset(gob8f[:, :, H + 1:MPAD], 0.0)

        # ---------------- prologue pieces ----------------
        # self-exp fold helper state: bounce selfexp through HBM for a
        # partition-broadcast read (engines cannot partition-broadcast).
        sxp_hbm = nc.dram_tensor("sxp_hbm", [NS], BF16, kind="Internal").ap()
        sxp = big.tile([H, NS], BF16)

        def map_chunk_a(c):
            """map_q (duplicated) for map cols [c*512, ..)."""
            sl = slice(c * GW, (c + 1) * GW)
            q_ps = ps.tile([P, 2, GW], F32, tag="st", name=f"qps{c}")
            nc.tensor.matmul(q_ps[:, 0, :], wq2, mapT[:, sl],
                             start=True, stop=True)
            nc.vector.tensor_copy(qT[:, sl], q_ps[:, 0, :])

        def map_chunk_b(c):
            """map_k, self-score, selfexp for map cols [c*512, ..)."""
            sl = slice(c * GW, (c + 1) * GW)
            k_ps = ps.tile([P, 2, GW], F32, tag="st", name=f"kps{c}")
            nc.tensor.matmul(k_ps[:, 0, :], wk2, mapT[:, sl],
                             start=True, stop=True)
            qk = sb_sm.tile([E, GW], BF16, tag="qk", name=f"qk{c}")
            nc.vector.tensor_tensor(out=qk, in0=qT[0:E, sl],
                                    in1=k_ps[0:E, 0, :], op=ALU.mult)
            # self-score sum lands in the unused upper half of k_ps
            ss_ps = k_ps[0:1, 1, :]
            nc.tensor.matmul(ss_ps, ones64, qk, start=True, stop=True)
            nc.scalar.activation(gmT[H:H + 1, sl], ss_ps, AF.Exp,
                                 scale=1.0 / TEMP, bias=msh[0:1])

        def map_chunk_c(c):
            """glu(map_v) for map cols [c*512, ..)."""
            sl = slice(c * GW, (c + 1) * GW)
            v_ps = ps.tile([P, 2, GW], F32, tag="st", name=f"vps{c}")
            nc.tensor.matmul(v_ps[0:E, 0, :], wv, mapT[:, sl],
                             start=True, stop=True)
            th = sb_sm.tile([H, GW], F32, tag="th", name=f"th{c}")
            nc.scalar.activation(th, v_ps[H:E, 0, :], AF.Tanh, scale=0.5)
            nc.gpsimd.tensor_scalar(out=th, in0=th, scalar1=0.5, scalar2=0.5,
                                    op0=ALU.mult, op1=ALU.add)
            nc.vector.tensor_tensor(out=gmT[0:H, sl], in0=v_ps[0:H, 0, :],
                                    in1=th, op=ALU.mult)

        def sxp_fold(c):
            sl = slice(c * GW, (c + 1) * GW)
            nc.sync.dma_start(sxp_hbm[sl], gmT[H:H + 1, sl])
            nc.sync.dma_start(sxp[:, sl], _bc_part(sxp_hbm[sl], H))
            nc.vector.tensor_tensor(out=gmT[0:H, sl], in0=gmT[0:H, sl],
                                    in1=sxp[:, sl], op=ALU.mult)

        def okS_chunk2(c2, eng="v"):
            """obs_k.T (dup halves) for obs cols [c2*1024, ..): 2 matmuls
            into one PSUM tile, ONE bulk cast out. No parity shuffling --
            the ST matmuls slice [0:64] x even cols / [64:128] x odd."""
            k_ps = ps.tile([P, 2, GW], F32, tag="st", name=f"okps{c2}")
            for t in range(2):
                c = 2 * c2 + t
                sl = slice(c * GW, (c + 1) * GW)
                nc.tensor.matmul(k_ps[:, t, :], wk2, obsT[:, sl],
                                 start=True, stop=True)
            dst = okS[:, c2 * 2 * GW:(c2 + 1) * 2 * GW]
            src = k_ps.rearrange("p a b -> p (a b)")
            if eng == "v":
                nc.vector.tensor_copy(dst, src)
            else:
                nc.scalar.copy(dst, src)

        def obs_v_batch2(c2, nb=16):
            """glu(obs_v) for nb consecutive obs blocks (one PSUM alloc)."""
            v_ps = ps.tile([P, 16, E], F32, tag="st", name=f"ovps{c2}")
            for b in range(nb):
                blk = c2 * 16 + b
                nc.tensor.matmul(v_ps[:, b, :],
                                 obsT[:, blk * P:(blk + 1) * P], wv,
                                 start=True, stop=True)
            tho = sb_sm.tile([P, 16, H], F32, tag="tho", name=f"tho{c2}")
            nc.scalar.activation(tho[:, 0:nb, :], v_ps[:, 0:nb, H:E],
                                 AF.Tanh, scale=0.5)
            nc.gpsimd.tensor_scalar(out=tho[:, 0:nb, :], in0=tho[:, 0:nb, :],
                                    scalar1=0.5, scalar2=0.5,
                                    op0=ALU.mult, op1=ALU.add)
            # blocks 16*c2.. -> pairs 8*c2.., t = parity
            og = gob8[:, 8 * c2:8 * c2 + nb // 2, :, 0:H]
            vi = v_ps[:, 0:nb, 0:H].rearrange("p (a b) h -> p a b h", b=2)
            ti = tho[:, 0:nb, :].rearrange("p (a b) h -> p a b h", b=2)
            nc.vector.tensor_tensor(out=og, in0=vi, in1=ti, op=ALU.mult)

        def map_pb_fill():
            bo_rep = bass.AP(tensor=bo_b.tensor, offset=bo_b.offset,
                             ap=[list(bo_b.ap[0]), [0, NT], [1, E]])
            nc.gpsimd.tensor_tensor(out=map_pb, in0=map_rows, in1=bo_rep,
                                    op=ALU.add)

        def agg_flush(g, agg, eng="v"):
            sl = slice(g * GW, (g + 1) * GW)
            if eng == "v":
                nc.vector.tensor_copy(ags[0:H + 1, sl], agg[0:H + 1, :])
            else:
                nc.scalar.copy(ags[0:H + 1, sl], agg[0:H + 1, :])

        # ---------------- epilogue ----------------
        epi_state = {}

        def epi_uda(half):
            """PE matmuls + PSUM evacuation + denominators for one half."""
            base = half * (NT // 2)
            uda = ps.tile([P, 8, P], F32, tag="st", name=f"uda{half}")
            for i in range(8):
                sl = slice((base + i) * P, (base + i + 1) * P)
                nc.tensor.matmul(uda[:, i, 0:E + 2], ags[:, sl], woe,
                                 start=True, stop=True)
            uds = sb_sm.tile([P, 8, E + 2], F32, tag="uds", name=f"uds{half}")
            nc.vector.tensor_copy(uds, uda[:, :, 0:E + 2])
            rden = sb_sm.tile([P, 8], F32, tag="rden", name=f"rden{half}")
            nc.vector.reciprocal(rden, uds[:, :, E])
            epi_state[half] = (uds, rden)

        def epi_stats(half, i0, i1):
            """out_pre + bn stats for tiles [base+i0, base+i1)."""
            base = half * (NT // 2)
            uds, rden = epi_state[half]
            for i in range(i0, i1):
                t = base + i
                nc.vector.scalar_tensor_tensor(out=out_pre[:, t, :],
                                               in0=uds[:, i, 0:E],
                                               scalar=rden[:, i:i + 1],
                                               in1=map_pb[:, t, :],
                                               op0=ALU.mult, op1=ALU.add)
                stats = sb_sm.tile([P, 6], F32, tag="stats", name=f"stats{t}")
                nc.vector.bn_stats(stats, out_pre[:, t, :])
                nc.vector.bn_aggr(mvC[:, t, :], stats)

        def epi_half(half):
            epi_uda(half)
            epi_stats(half, 0, 8)

        def epi_rstd(half):
            """1/sqrt(var+eps) for 8 tiles via min-poly + 2 NR steps."""
            tsl = slice(half * (NT // 2), (half + 1) * (NT // 2))
            w = NT // 2
            vpe = sb_sm.tile([P, w], F32, tag="vpe", name=f"vpe{half}")
            nc.vector.tensor_scalar_add(vpe, mvC[:, tsl, 1], EPS)
            c1 = sb_sm.tile([P, w], F32, tag="nc1", name=f"nc1{half}")
            nc.vector.tensor_scalar(out=c1, in0=vpe, scalar1=0.564185,
                                    scalar2=0.378467, op0=ALU.mult,
                                    op1=ALU.add)
            c2 = sb_sm.tile([P, w], F32, tag="nc2", name=f"nc2{half}")
            nc.vector.tensor_scalar(out=c2, in0=vpe, scalar1=0.288949,
                                    scalar2=0.791321, op0=ALU.mult,
                                    op1=ALU.add)
            nc.vector.tensor_tensor(out=c1, in0=c1, in1=c2, op=ALU.min)
            rs = rstd[:, tsl]
            nc.vector.reciprocal(rs, c1)
            for _ in range(2):
                nc.vector.tensor_tensor(out=c1, in0=rs, in1=rs, op=ALU.mult)
                nc.vector.tensor_tensor(out=c1, in0=c1, in1=vpe, op=ALU.mult)
                nc.vector.tensor_scalar(out=c1, in0=c1, scalar1=-0.5,
                                        scalar2=1.5, op0=ALU.mult,
                                        op1=ALU.add)
                nc.vector.tensor_tensor(out=rs, in0=rs, in1=c1, op=ALU.mult)

        def epi_xn(half, i0, i1):
            """DVE normalize tiles [base+i0, base+i1) into out_all."""
            base = half * (NT // 2)
            for t in range(base + i0, base + i1):
                nc.vector.tensor_scalar(out=out_all[:, t, :],
                                        in0=out_pre[:, t, :],
                                        scalar1=mvC[:, t, 0:1],
                                        scalar2=rstd[:, t:t + 1],
                                        op0=ALU.subtract, op1=ALU.mult)

        def epi_out(half):
            """gamma/beta (2 big gpsimd broadcast ops) + output DMA."""
            t0, t1 = half * (NT // 2), (half + 1) * (NT // 2)
            ga_rep = bass.AP(tensor=ga_b.tensor, offset=ga_b.offset,
                             ap=[list(ga_b.ap[0]), [0, t1 - t0], [1, E]])
            be_rep = bass.AP(tensor=be_b.tensor, offset=be_b.offset,
                             ap=[list(be_b.ap[0]), [0, t1 - t0], [1, E]])
            nc.gpsimd.tensor_tensor(out=out_all[:, t0:t1, :],
                                    in0=out_all[:, t0:t1, :],
                                    in1=ga_rep, op=ALU.mult)
            nc.gpsimd.tensor_tensor(out=out_all[:, t0:t1, :],
                                    in0=out_all[:, t0:t1, :],
                                    in1=be_rep, op=ALU.add)
            od = out_d.rearrange("(t p) e -> p t e", p=P)
            for q in range(2):
                qsl = slice(t0 + q * (NT // 4), t0 + (q + 1) * (NT // 4))
                nc.sync.dma_start(od[:, qsl, :], out_all[:, qsl, :])

        # ---------------- prologue head (lean) ----------------
        # Only what hp0's first pairs need: map chunks 0,1 (group cols),
        # okS chunks 0,1 (pairs 0-3), gob batch 0 (pairs 0-7).
        map_chunk_a(0)
        map_chunk_b(0)
        map_chunk_c(0)
        sxp_fold(0)
        okS_chunk2(0)
        map_chunk_a(1)
        map_chunk_b(1)
        map_chunk_c(1)
        sxp_fold(1)
        obs_v_batch2(0)

        # drip schedule for hp0: item lists per pair index.
        # okS chunk c2 must land before pair 4*c2; gob batch b before
        # pair 8*b; map chunks 2,3 (qT/gmT for hp1) anywhere before hp1.
        drip = {
            0: [lambda: okS_chunk2(1)],
            2: [lambda: okS_chunk2(2)],
            4: [lambda: obs_v_batch2(1)],
            6: [lambda: okS_chunk2(3)],
            8: [lambda: okS_chunk2(4)],
            10: [lambda: obs_v_batch2(2)],
            12: [lambda: okS_chunk2(5), lambda: map_pb_fill()],
            14: [lambda: okS_chunk2(6)],
            16: [lambda: obs_v_batch2(3)],
            18: [lambda: okS_chunk2(7)],
            20: [lambda: map_chunk_a(2)],
            21: [lambda: map_chunk_b(2)],
            22: [lambda: map_chunk_c(2)],
            23: [lambda: sxp_fold(2)],
            24: [lambda: map_chunk_a(3)],
            25: [lambda: map_chunk_b(3)],
            26: [lambda: map_chunk_c(3)],
            27: [lambda: sxp_fold(3)],
        }

        def exp_unit(st_t, pt_t, eng):
            if eng == "a":
                nc.scalar.activation(pt_t, st_t, AF.Exp,
                                     scale=1.0 / TEMP, bias=msh)
            else:
                nc.vector.tensor_scalar(out=pt_t.bitcast(U8), in0=st_t,
                                        scalar1=SCH_A, scalar2=SCH_B,
                                        op0=ALU.mult, op1=ALU.add)

        # ---------------- main loop: 2 half-passes x 32 pairs ----------
        # Software-pipelined by one pair: the PV for pair p-1 is issued to
        # the PE AFTER pair p's ST matmuls, so by the time the PE FIFO
        # reaches it, exp(p-1) has long finished.
        for hp in range(2):
            agg0 = ps_agg.tile([MPAD, GW], F32, tag="agg", name=f"agg{hp}_0")
            agg1 = ps_agg.tile([MPAD, GW], F32, tag="agg", name=f"agg{hp}_1")
            g0 = 2 * hp
            g1 = 2 * hp + 1
            s0 = slice(g0 * GW, (g0 + 1) * GW)
            s1 = slice(g1 * GW, (g1 + 1) * GW)
            nc.tensor.matmul(agg0, id33, gmT[:, s0],
                             start=True, stop=False)
            nc.tensor.matmul(agg1, id33, gmT[:, s1],
                             start=True, stop=False)
            # hp1 embedded epilogue schedule for half 0 (spread into small
            # sub-blobs so the DVE backlog never starves the exp->PV chain)
            epi_sched = {} if hp == 0 else {
                5: [lambda: epi_uda(0)],
                6: [lambda: epi_stats(0, 0, 4)],
                8: [lambda: epi_stats(0, 4, 8)],
                10: [lambda: epi_rstd(0)],
                13: [lambda: epi_xn(0, 0, 4)],
                15: [lambda: epi_xn(0, 4, 8), lambda: epi_out(0)],
            }
            pv_q = []
            for pp in range(NPAIR):
                co = 256 * pp
                ko_lo = okS[0:E, co:co + P]
                ko_hi = okS[E:P, co + P:co + 2 * P]
                st0 = ps.tile([P, 2, GW], F32, tag="st", name=f"st{hp}_{pp}_0")
                st1 = ps.tile([P, 2, GW], F32, tag="st", name=f"st{hp}_{pp}_1")
                nc.tensor.matmul(st0[:, 0, :], ko_lo, qT[0:E, s0],
                                 start=True, stop=True)
                nc.tensor.matmul(st0[:, 1, :], ko_hi, qT[E:P, s0],
                                 start=True, stop=True)
                nc.tensor.matmul(st1[:, 0, :], ko_lo, qT[0:E, s1],
                                 start=True, stop=True)
                nc.tensor.matmul(st1[:, 1, :], ko_hi, qT[E:P, s1],
                                 start=True, stop=True)
                # PV pipelined TWO pairs deep: exp(p) has ~2 pair-times of
                # slack before the PE would ever wait on it.
                if len(pv_q) >= 2:
                    qq, qt0, qt1 = pv_q.pop(0)
                    go = gob8[:, qq, :, :]
                    nc.tensor.matmul(agg0, go, qt0, start=False, stop=False,
                                     perf_mode=DR)
                    nc.tensor.matmul(agg1, go, qt1, start=False, stop=False,
                                     perf_mode=DR)
                pt0 = sb_pt.tile([P, 2, GW], FP8E5, tag="pt",
                                 name=f"pt{hp}_{pp}_0")
                pt1 = sb_pt.tile([P, 2, GW], FP8E5, tag="pt",
                                 name=f"pt{hp}_{pp}_1")
                # exp split: mostly 1:1 ACT/DVE with a few ACT "bonus"
                # pairs to offset DVE's drip (hp0) / epilogue (hp1) load.
                bonus = pp in ((10, 21) if hp == 0 else (6, 9, 12))
                exp_unit(st0, pt0, "a")
                exp_unit(st1, pt1, "a" if bonus else "v")
                pv_q.append((pp, pt0, pt1))
                if hp == 0:
                    for fn in drip.get(pp, ()):
                        fn()
                else:
                    for fn in epi_sched.get(pp, ()):
                        fn()
            while pv_q:
                qq, qt0, qt1 = pv_q.pop(0)
                go = gob8[:, qq, :, :]
                last = not pv_q
                nc.tensor.matmul(agg0, go, qt0, start=False, stop=last,
                                 perf_mode=DR)
                nc.tensor.matmul(agg1, go, qt1, start=False, stop=last,
                                 perf_mode=DR)
            agg_flush(g0, agg0, eng="v")
            agg_flush(g1, agg1, eng="a")

        # ---------------- epilogue tail (half 1) ----------
        epi_half(1)
        epi_rstd(1)
        epi_xn(1, 0, 8)
        epi_out(1)

        if dbg is not None:
            nc.sync.dma_start(dbg["qT"], qT)
            nc.sync.dma_start(dbg["gmT"], gmT)
            nc.sync.dma_start(dbg["ags"], ags)
            nc.sync.dma_start(dbg["okS"], okS)
            nc.sync.dma_start(dbg["gob8"],
                              gob8.rearrange("p a b c -> p (a b c)"))
            nc.sync.dma_start(dbg["out_pre"],
                              out_pre.rearrange("p a b -> p (a b)"))
            nc.sync.dma_start(dbg["mvC"], mvC.rearrange("p a b -> p (a b)"))


_CACHED = None


def _build(debug=False):
    global _CACHED
    if _CACHED is not None and not debug:
        return _CACHED
    nc = bacc.Bacc("TRN2", target_bir_lowering=False, debug=False)

    def din(name, shape, dt=F32):
        return nc.dram_tensor(name, shape, dt, kind="ExternalInput").ap()

    map_rows_d = din("map_rows", [NS, E])
    mapT_d = din("mapT", [E, NS], BF16)
    obsT_d = din("obsT", [E, NO], BF16)
    wpb_d = din("wpb", [E, BW], BF16)
    wpf_d = din("wpf", [E, FW], F32R)
    vec_d = din("vpack", [3 * E + 1])
    out_d = nc.dram_tensor("out", [NS, E], F32, kind="ExternalOutput").ap()

    dbg = None
    if debug:
        def dout(name, shape, dt=F32):
            return nc.dram_tensor(name, shape, dt, kind="ExternalOutput").ap()
        dbg = {
            "qT": dout("dbg_qT", [P, NS], BF16),
            "gmT": dout("dbg_gmT", [H + 1, NS], BF16),
            "ags": dout("dbg_ags", [H + 1, NS], F32R),
            "okS": dout("dbg_okS", [P, NO], BF16),
            "gob8": dout("dbg_gob8", [P, NPAIR * 2 * MPAD], FP8),
            "out_pre": dout("dbg_out_pre", [P, NT * E]),
            "mvC": dout("dbg_mvC", [P, NT * 2]),
        }

    with tile.TileContext(nc) as tc:
        _emit(tc, out_d, map_rows_d, mapT_d, obsT_d, wpb_d, wpf_d,
              vec_d, dbg=dbg)
    nc.compile()
    if not debug:
        _CACHED = nc
    return nc


def _prep_in_maps(map_code, obs_code, Wq, Wk, Wv, Wo, bo, gamma, beta):
    f = np.float32
    map_code = np.ascontiguousarray(np.asarray(map_code, dtype=f))
    obs_code = np.asarray(obs_code, dtype=f)

    bf16_np = mybir.dt.np(BF16)

    def to_bf16(x):
        return np.ascontiguousarray(np.asarray(x, dtype=f).astype(bf16_np))

    obsT = np.ascontiguousarray(obs_code.T)

    wq2 = np.concatenate([np.asarray(Wq, f), np.asarray(Wq, f)], axis=1)
    wk2 = np.concatenate([np.asarray(Wk, f), np.asarray(Wk, f)], axis=1)
    wpb = np.zeros((E, BW), dtype=f)
    wpb[:, _WQ0:_WQ0 + 128] = wq2
    wpb[:, _WK0:_WK0 + 128] = wk2
    wpb[:, _WV0:_WV0 + E] = np.asarray(Wv, f)
    wpb[:, _ONES0] = 1.0
    for k in range(H + 1):
        wpb[k, _IDO + k] = 1.0   # identity seed stationary [33, MPAD]

    woe = np.zeros((E, FW), dtype=f)
    woe[0:H, 0:E] = np.asarray(Wo, dtype=f)
    woe[H, E] = 1.0

    vpack = np.concatenate([
        np.asarray(bo, dtype=f), np.asarray(gamma, dtype=f),
        np.asarray(beta, dtype=f), np.full((1,), -SHIFT, dtype=f),
    ])
    shared = {
        "obsT": to_bf16(obsT),
        "wpb": to_bf16(wpb),
        "wpf": np.ascontiguousarray(woe),
        "vpack": np.ascontiguousarray(vpack),
    }
    in_maps = []
    for i in range(NCORES):
        shard = map_code[i * NS:(i + 1) * NS]
        m = dict(shared)
        m["map_rows"] = shard
        m["mapT"] = to_bf16(np.ascontiguousarray(shard.T))
        in_maps.append(m)
    return in_maps


def run(trace=False, **inputs):
    nc = _build()
    in_maps = _prep_in_maps(**inputs)
    res = run_bass_kernel_spmd(nc, in_maps, list(range(NCORES)), trace=trace)
    out = np.concatenate([res.results[i]["out"] for i in range(NCORES)], axis=0)
    return out, res


def kernel(**inputs):
    out, _ = run(trace=False, **inputs)
    return out


# revision 17
# speedup vs baseline: 1.1579x; 1.0120x over previous
"""Trainium2 Bass kernel for nn_Attention_5815385719367 (gnn_message_passing).

Computation (see reference):
  map_q/k/v = map_code @ Wq/Wk/Wv ; obs_k/v = obs_code @ Wk/Wv
  scores    = [sum(q*k,-1) | q @ obs_k.T] / 8
  w         = softmax(scores)
  agg       = w[:, :1]*glu(map_v) + w[:, 1:] @ glu(obs_v)
  out       = LN(agg @ Wo + bo + map_code) * gamma + beta

Sharding: data-parallel over N_map rows (2048 rows/core x 8 cores);
obs_code and weights replicated. No collectives.

v3 design notes (per core), building on v2:
  - scores computed TRANSPOSED in PSUM: ST[obs=128, map] via PE ROW-TILED
    pairs: even obs block's k.T sits on SBUF partitions 0-63 (PE tile
    (0,0)), odd block's on partitions 64-127 (tile (64,0)).
  - v3: qT/okS/gmT/ones/id33 are BF16 (v2 used f32r). f32 moving
    operands stream the PE at 2 cyc/col; bf16 streams 1 cyc/col, so the
    ST matmuls halve (427 -> ~220 ns each measured).
  - v3: okS keeps the k.T projection in its natural [128, NO] layout
    (dup halves from the wk|wk projection); the even/odd parity split
    is done by SLICING (partitions 0:64 x even cols / 64:128 x odd
    cols) -- v2's per-block parity copies were pure waste.
  - v3: a ~4us junk-matmul warmup burst runs during the input DMA so
    the PE HAM clock-gate opens (1.2 -> 2.4 GHz) before real work; v2
    ran the whole prologue + 10 pairs cold.
  - v3: lean prologue -- only map chunks 0,1 (hp0's groups), okS chunk
    pair 0 and gob batch 0 precede the main loop; map chunks 2,3 (only
    needed by hp1) and the rest of okS/gob drip into hp0.
  - softmax exp is the hard wall (1 elem/lane/cycle on ACT): split it
    between ACT (direct exp -> fp8 e5m2, logits shifted by +2) and DVE
    (Schraudolph: one mult-add tensor_scalar into uint8 whose bit
    pattern IS the e5m2 log-domain approximation).
  - PV runs fp8 DoubleRow over block PAIRS: stationary
    gob8[128, 2, 80] = glu(obs_v)|ones|zero-pad, moving pt8; the ones
    column accumulates the softmax denominator for free.
  - self-attention term folded into the PV accumulator seed via an
    identity-33 stationary matmul.
  - v3: tail epilogue (half 1) uses DVE for the normalize step and two
    big broadcast tensor_tensor ops for gamma/beta instead of 16
    small per-tile ops.
"""

import numpy as np

import concourse.bass as bass
import concourse.bacc as bacc
import concourse.tile as tile
from concourse import mybir
from concourse.bass_utils import run_bass_kernel_spmd

NCORES = 8
NM, NO, E = 16384, 8192, 64
NS = NM // NCORES            # 2048 map rows per core
H = E // 2                   # 32
TEMP = 8.0
EPS = 1e-6
P = 128
NT = NS // P                 # 16 row tiles per core
GW = 512                     # map group width (psum bank)
NPAIR = NO // 256            # 32 obs block-pairs
SHIFT = -2.0                 # logit shift: exp(l - SHIFT), cancels in ratio.
MPAD = 80                    # padded PV output partitions (33 real; %16 keeps
                             # the DoubleRow ldweights step legal, >64 keeps
                             # column tiling off)

F32 = mybir.dt.float32
F32R = mybir.dt.float32r
BF16 = mybir.dt.bfloat16
FP8 = mybir.dt.float8e4
FP8E5 = mybir.dt.float8e5
U8 = mybir.dt.uint8
AF = mybir.ActivationFunctionType
ALU = mybir.AluOpType
DR = mybir.MatmulPerfMode.DoubleRow

# Schraudolph constants for uint8 e5m2 log-domain exp of RAW score s:
#   i = 4*log2(exp(s/8 - SHIFT)) + 60 - sawtooth_center
SCH_A = 4.0 * 1.4426950408889634 / TEMP   # 0.72135
SCH_B = 60.0 - 4.0 * 1.4426950408889634 * SHIFT - 0.229

# layout of the bf16 weight pack [64, BW]
_WQ0 = 0              # wq duplicated [64, 128]
_WK0 = 128            # wk duplicated [64, 128]
_WV0 = 256            # wv [64, 64]
_ONES0 = 320          # ones column [64, 1]
_IDO = 321            # identity [33, MPAD] seed stationary
BW = 321 + MPAD

# layout of the f32r pack [64, FW]: woe only
FW = E + 2


def _bc_part(ap, n):
    """Broadcast a [x, ...] AP along a new leading partition dim of n."""
    return bass.AP(tensor=ap.tensor, offset=ap.offset, ap=[[0, n]] + list(ap.ap))


def _emit(tc, out_d, map_rows_d, mapT_d, obsT_d, wpb_d, wpf_d, vec_d,
          wup_d, dbg=None):
    nc = tc.nc
    with tc.tile_pool(name="consts", bufs=1) as consts, \
         tc.tile_pool(name="big", bufs=1) as big, \
         tc.tile_pool(name="sb_sm", bufs=3) as sb_sm, \
         tc.tile_pool(name="sb_pt", bufs=6) as sb_pt, \
         tc.tile_pool(name="ps", bufs=3, space="PSUM") as ps, \
         tc.tile_pool(name="ps_agg", bufs=2, space="PSUM") as ps_agg:

        # ---------------- constants ----------------
        wpb = consts.tile([E, BW], BF16)          # bf16 weights pack
        nc.sync.dma_start(wpb, wpb_d)
        wq2 = wpb[:, _WQ0:_WQ0 + 128]             # [64,128] wq|wq
        wk2 = wpb[:, _WK0:_WK0 + 128]             # [64,128] wk|wk
        wv = wpb[:, _WV0:_WV0 + E]                # [64,64]
        ones64 = wpb[:, _ONES0:_ONES0 + 1]
        id33 = wpb[0:H + 1, _IDO:_IDO + MPAD]     # identity seed [33, 80]

        vecs = consts.tile([P, 3 * E + 1], F32)   # bo|gamma|beta|-shift
        nc.sync.dma_start(vecs, _bc_part(vec_d, P))
        bo_b = vecs[:, 0:E]
        ga_b = vecs[:, E:2 * E]
        be_b = vecs[:, 2 * E:3 * E]
        msh = vecs[:, 3 * E:3 * E + 1]            # -SHIFT bias column

        wpf = consts.tile([E, FW], F32R)
        nc.sync.dma_start(wpf, wpf_d)
        woe = wpf[0:H + 1, 0:E + 2]               # [33,66]

        # ---------------- warmup: open the HAM clock gate --------------
        # ~26 junk matmuls (~7us cold) with a FULL 128-row stationary --
        # K=64 warmups never tripped the activity monitor (only half the
        # row groups active); the flip was always observed once the
        # full-array DR PVs started. Output PSUM is never read.
        wup = consts.tile([P, GW], BF16)
        nc.sync.dma_start(wup, wup_d)
        for wi in range(26):
            wu = ps.tile([P, 2, GW], F32, tag="st", name=f"wu{wi}")
            nc.tensor.matmul(wu[:, 0, 0:320], wup[:, 0:P], wup[:, 0:320],
                             start=True, stop=True)

        # ---------------- big arenas + input DMAs ----------------
        # NOTE: every dma_start dispatch costs ~600ns SERIAL time on the
        # issuing engine's queue -- keep the count low and the order
        # matched to consumption so the sxp_fold bounces (enqueued after
        # these) dispatch early.
        mapT = big.tile([E, NS], BF16)
        obsT = big.tile([E, NO], BF16)
        for lo, hi, t_, s_ in ((0, 512, mapT, mapT_d),
                               (0, 1024, obsT, obsT_d),
                               (512, 1024, mapT, mapT_d),
                               (1024, 2048, obsT, obsT_d),
                               (1024, 2048, mapT, mapT_d),
                               (2048, 8192, obsT, obsT_d)):
            nc.sync.dma_start(t_[:, lo:hi], s_[:, lo:hi])
        map_rows = big.tile([P, NT, E], F32)
        nc.scalar.dma_start(map_rows, map_rows_d.rearrange("(t p) e -> p t e", p=P))

        qT = big.tile([P, NS], BF16)              # map_q.T duplicated halves
        gmT = big.tile([H + 1, NS], BF16)         # [glu(map_v).T ; selfexp]
        okS = big.tile([P, NO], BF16)             # obs_k.T duplicated halves
        gob8 = big.tile([P, NPAIR, 2, MPAD], FP8)  # glu(obs_v)|1|0 pairs
        ags = big.tile([H + 1, NS], F32R)         # [numer.T ; denom]
        map_pb = big.tile([P, NT, E], F32)        # map + bo
        out_pre = big.tile([P, NT, E], F32)
        out_all = big.tile([P, NT, E], F32)
        mvC = big.tile([P, NT, 2], F32)
        rstd = big.tile([P, NT], F32)

        # gob8 static columns: ones at h=32, zeros at h=33..79. memset on
        # the (idle) gpsimd -- the v2/v3 broadcast-DMA version cost ~10us
        # of serial dispatch on the sync queue.
        gob8f = gob8.rearrange("p a b c -> p (a b) c")
        nc.gpsimd.memset(gob8f[:, :, H:H + 1], 1.0)
        nc.gpsimd.memset(gob8f[:, :, H + 1:MPAD], 0.0)

        # ---------------- prologue pieces ----------------
        # self-exp fold helper state: bounce selfexp through HBM for a
        # partition-broadcast read (engines cannot partition-broadcast).
        sxp_hbm = nc.dram_tensor("sxp_hbm", [NS], BF16, kind="Internal").ap()
        sxp = big.tile([H, NS], BF16)

        def map_chunk_a(c):
            """map_q (duplicated) for map cols [c*512, ..)."""
            sl = slice(c * GW, (c + 1) * GW)
            q_ps = ps.tile([P, 2, GW], F32, tag="st", name=f"qps{c}")
            nc.tensor.matmul(q_ps[:, 0, :], wq2, mapT[:, sl],
                             start=True, stop=True)
            nc.vector.tensor_copy(qT[:, sl], q_ps[:, 0, :])

        def map_chunk_b(c):
            """map_k, self-score, selfexp for map cols [c*512, ..)."""
            sl = slice(c * GW, (c + 1) * GW)
            k_ps = ps.tile([P, 2, GW], F32, tag="st", name=f"kps{c}")
            nc.tensor.matmul(k_ps[:, 0, :], wk2, mapT[:, sl],
                             start=True, stop=True)
            qk = sb_sm.tile([E, GW], BF16, tag="qk", name=f"qk{c}")
            nc.vector.tensor_tensor(out=qk, in0=qT[0:E, sl],
                                    in1=k_ps[0:E, 0, :], op=ALU.mult)
            # self-score sum lands in the unused upper half of k_ps
            ss_ps = k_ps[0:1, 1, :]
            nc.tensor.matmul(ss_ps, ones64, qk, start=True, stop=True)
            nc.scalar.activation(gmT[H:H + 1, sl], ss_ps, AF.Exp,
                                 scale=1.0 / TEMP, bias=msh[0:1])

        def map_chunk_c(c):
            """glu(map_v) for map cols [c*512, ..)."""
            sl = slice(c * GW, (c + 1) * GW)
            v_ps = ps.tile([P, 2, GW], F32, tag="st", name=f"vps{c}")
            nc.tensor.matmul(v_ps[0:E, 0, :], wv, mapT[:, sl],
                             start=True, stop=True)
            th = sb_sm.tile([H, GW], F32, tag="th", name=f"th{c}")
            nc.scalar.activation(th, v_ps[H:E, 0, :], AF.Tanh, scale=0.5)
            nc.gpsimd.tensor_scalar(out=th, in0=th, scalar1=0.5, scalar2=0.5,
                                    op0=ALU.mult, op1=ALU.add)
            nc.vector.tensor_tensor(out=gmT[0:H, sl], in0=v_ps[0:H, 0, :],
                                    in1=th, op=ALU.mult)

        def sxp_fold(c):
            sl = slice(c * GW, (c + 1) * GW)
            nc.sync.dma_start(sxp_hbm[sl], gmT[H:H + 1, sl])
            nc.sync.dma_start(sxp[:, sl], _bc_part(sxp_hbm[sl], H))
            nc.vector.tensor_tensor(out=gmT[0:H, sl], in0=gmT[0:H, sl],
                                    in1=sxp[:, sl], op=ALU.mult)

        def okS_chunk2(c2, eng="v"):
            """obs_k.T (dup halves) for obs cols [c2*1024, ..): 2 matmuls
            into one PSUM tile, ONE bulk cast out. No parity shuffling --
            the ST matmuls slice [0:64] x even cols / [64:128] x odd."""
            k_ps = ps.tile([P, 2, GW], F32, tag="st", name=f"okps{c2}")
            for t in range(2):
                c = 2 * c2 + t
                sl = slice(c * GW, (c + 1) * GW)
                nc.tensor.matmul(k_ps[:, t, :], wk2, obsT[:, sl],
                                 start=True, stop=True)
            dst = okS[:, c2 * 2 * GW:(c2 + 1) * 2 * GW]
            src = k_ps.rearrange("p a b -> p (a b)")
            if eng == "v":
                nc.vector.tensor_copy(dst, src)
            else:
                nc.scalar.copy(dst, src)

        def obs_v_batch2(c2, nb=16):
            """glu(obs_v) for nb consecutive obs blocks (one PSUM alloc)."""
            v_ps = ps.tile([P, 16, E], F32, tag="st", name=f"ovps{c2}")
            for b in range(nb):
                blk = c2 * 16 + b
                nc.tensor.matmul(v_ps[:, b, :],
                                 obsT[:, blk * P:(blk + 1) * P], wv,
                                 start=True, stop=True)
            tho = sb_sm.tile([P, 16, H], F32, tag="tho", name=f"tho{c2}")
            nc.scalar.activation(tho[:, 0:nb, :], v_ps[:, 0:nb, H:E],
                                 AF.Tanh, scale=0.5)
            nc.gpsimd.tensor_scalar(out=tho[:, 0:nb, :], in0=tho[:, 0:nb, :],
                                    scalar1=0.5, scalar2=0.5,
                                    op0=ALU.mult, op1=ALU.add)
            # blocks 16*c2.. -> pairs 8*c2.., t = parity
            og = gob8[:, 8 * c2:8 * c2 + nb // 2, :, 0:H]
            vi = v_ps[:, 0:nb, 0:H].rearrange("p (a b) h -> p a b h", b=2)
            ti = tho[:, 0:nb, :].rearrange("p (a b) h -> p a b h", b=2)
            nc.vector.tensor_tensor(out=og, in0=vi, in1=ti, op=ALU.mult)

        def map_pb_fill():
            bo_rep = bass.AP(tensor=bo_b.tensor, offset=bo_b.offset,
                             ap=[list(bo_b.ap[0]), [0, NT], [1, E]])
            nc.gpsimd.tensor_tensor(out=map_pb, in0=map_rows, in1=bo_rep,
                                    op=ALU.add)

        def agg_flush(g, agg, eng="v"):
            sl = slice(g * GW, (g + 1) * GW)
            if eng == "v":
                nc.vector.tensor_copy(ags[0:H + 1, sl], agg[0:H + 1, :])
            else:
                nc.scalar.copy(ags[0:H + 1, sl], agg[0:H + 1, :])

        # ---------------- epilogue ----------------
        epi_state = {}

        def epi_uda(half):
            """PE matmuls + PSUM evacuation + denominators for one half."""
            base = half * (NT // 2)
            uda = ps.tile([P, 8, P], F32, tag="st", name=f"uda{half}")
            for i in range(8):
                sl = slice((base + i) * P, (base + i + 1) * P)
                nc.tensor.matmul(uda[:, i, 0:E + 2], ags[:, sl], woe,
                                 start=True, stop=True)
            uds = sb_sm.tile([P, 8, E + 2], F32, tag="uds", name=f"uds{half}")
            nc.vector.tensor_copy(uds, uda[:, :, 0:E + 2])
            rden = sb_sm.tile([P, 8], F32, tag="rden", name=f"rden{half}")
            nc.vector.reciprocal(rden, uds[:, :, E])
            epi_state[half] = (uds, rden)

        def epi_stats(half, i0, i1):
            """out_pre + bn stats for tiles [base+i0, base+i1)."""
            base = half * (NT // 2)
            uds, rden = epi_state[half]
            for i in range(i0, i1):
                t = base + i
                nc.vector.scalar_tensor_tensor(out=out_pre[:, t, :],
                                               in0=uds[:, i, 0:E],
                                               scalar=rden[:, i:i + 1],
                                               in1=map_pb[:, t, :],
                                               op0=ALU.mult, op1=ALU.add)
                stats = sb_sm.tile([P, 6], F32, tag="stats", name=f"stats{t}")
                nc.vector.bn_stats(stats, out_pre[:, t, :])
                nc.vector.bn_aggr(mvC[:, t, :], stats)

        def epi_half(half):
            epi_uda(half)
            epi_stats(half, 0, 8)

        def epi_rstd(half):
            """1/sqrt(var+eps) for 8 tiles via min-poly + 2 NR steps."""
            tsl = slice(half * (NT // 2), (half + 1) * (NT // 2))
            w = NT // 2
            vpe = sb_sm.tile([P, w], F32, tag="vpe", name=f"vpe{half}")
            nc.vector.tensor_scalar_add(vpe, mvC[:, tsl, 1], EPS)
            c1 = sb_sm.tile([P, w], F32, tag="nc1", name=f"nc1{half}")
            nc.vector.tensor_scalar(out=c1, in0=vpe, scalar1=0.564185,
                                    scalar2=0.378467, op0=ALU.mult,
                                    op1=ALU.add)
            c2 = sb_sm.tile([P, w], F32, tag="nc2", name=f"nc2{half}")
            nc.vector.tensor_scalar(out=c2, in0=vpe, scalar1=0.288949,
                                    scalar2=0.791321, op0=ALU.mult,
                                    op1=ALU.add)
            nc.vector.tensor_tensor(out=c1, in0=c1, in1=c2, op=ALU.min)
            rs = rstd[:, tsl]
            nc.vector.reciprocal(rs, c1)
            for _ in range(2):
                nc.vector.tensor_tensor(out=c1, in0=rs, in1=rs, op=ALU.mult)
                nc.vector.tensor_tensor(out=c1, in0=c1, in1=vpe, op=ALU.mult)
                nc.vector.tensor_scalar(out=c1, in0=c1, scalar1=-0.5,
                                        scalar2=1.5, op0=ALU.mult,
                                        op1=ALU.add)
                nc.vector.tensor_tensor(out=rs, in0=rs, in1=c1, op=ALU.mult)

        def epi_xn(half, i0, i1):
            """DVE normalize tiles [base+i0, base+i1) into out_all."""
            base = half * (NT // 2)
            for t in range(base + i0, base + i1):
                nc.vector.tensor_scalar(out=out_all[:, t, :],
                                        in0=out_pre[:, t, :],
                                        scalar1=mvC[:, t, 0:1],
                                        scalar2=rstd[:, t:t + 1],
                                        op0=ALU.subtract, op1=ALU.mult)

        def epi_out(half, q0=0, q1=2):
            """gamma/beta (big gpsimd broadcast ops) + output DMA, one
            NT//4-tile quarter at a time so DMA overlaps the next gabe."""
            base = half * (NT // 2)
            od = out_d.rearrange("(t p) e -> p t e", p=P)
            for q in range(q0, q1):
                t0 = base + q * (NT // 4)
                t1 = base + (q + 1) * (NT // 4)
                ga_rep = bass.AP(tensor=ga_b.tensor, offset=ga_b.offset,
                                 ap=[list(ga_b.ap[0]), [0, t1 - t0], [1, E]])
                be_rep = bass.AP(tensor=be_b.tensor, offset=be_b.offset,
                                 ap=[list(be_b.ap[0]), [0, t1 - t0], [1, E]])
                nc.gpsimd.tensor_tensor(out=out_all[:, t0:t1, :],
                                        in0=out_all[:, t0:t1, :],
                                        in1=ga_rep, op=ALU.mult)
                nc.gpsimd.tensor_tensor(out=out_all[:, t0:t1, :],
                                        in0=out_all[:, t0:t1, :],
                                        in1=be_rep, op=ALU.add)
                nc.sync.dma_start(od[:, t0:t1, :], out_all[:, t0:t1, :])

        # ---------------- prologue head (lean) ----------------
        # Only what hp0's first pairs need: map chunks 0,1 (group cols),
        # okS chunks 0,1 (pairs 0-3), gob batch 0 (pairs 0-7).
        map_chunk_a(0)
        map_chunk_b(0)
        map_chunk_c(0)
        sxp_fold(0)
        okS_chunk2(0)
        map_chunk_a(1)
        map_chunk_b(1)
        map_chunk_c(1)
        sxp_fold(1)
        obs_v_batch2(0)

        # drip schedule for hp0: item lists per pair index.
        # okS chunk c2 must land before pair 4*c2; gob batch b before
        # pair 8*b; map chunks 2,3 (qT/gmT for hp1) anywhere before hp1.
        drip = {
            0: [lambda: okS_chunk2(1)],
            2: [lambda: okS_chunk2(2)],
            4: [lambda: obs_v_batch2(1)],
            6: [lambda: okS_chunk2(3)],
            8: [lambda: okS_chunk2(4)],
            10: [lambda: obs_v_batch2(2)],
            12: [lambda: okS_chunk2(5), lambda: map_pb_fill()],
            14: [lambda: okS_chunk2(6)],
            16: [lambda: obs_v_batch2(3)],
            18: [lambda: okS_chunk2(7)],
            20: [lambda: map_chunk_a(2)],
            21: [lambda: map_chunk_b(2)],
            22: [lambda: map_chunk_c(2)],
            23: [lambda: sxp_fold(2)],
            24: [lambda: map_chunk_a(3)],
            25: [lambda: map_chunk_b(3)],
            26: [lambda: map_chunk_c(3)],
            27: [lambda: sxp_fold(3)],
        }

        def exp_unit(st_t, pt_t, eng):
            if eng == "a":
                nc.scalar.activation(pt_t, st_t, AF.Exp,
                                     scale=1.0 / TEMP, bias=msh)
            else:
                nc.vector.tensor_scalar(out=pt_t.bitcast(U8), in0=st_t,
                                        scalar1=SCH_A, scalar2=SCH_B,
                                        op0=ALU.mult, op1=ALU.add)

        # ---------------- main loop: 2 half-passes x 32 pairs ----------
        # Software-pipelined by one pair: the PV for pair p-1 is issued to
        # the PE AFTER pair p's ST matmuls, so by the time the PE FIFO
        # reaches it, exp(p-1) has long finished.
        for hp in range(2):
            agg0 = ps_agg.tile([MPAD, GW], F32, tag="agg", name=f"agg{hp}_0")
            agg1 = ps_agg.tile([MPAD, GW], F32, tag="agg", name=f"agg{hp}_1")
            g0 = 2 * hp
            g1 = 2 * hp + 1
            s0 = slice(g0 * GW, (g0 + 1) * GW)
            s1 = slice(g1 * GW, (g1 + 1) * GW)
            nc.tensor.matmul(agg0, id33, gmT[:, s0],
                             start=True, stop=False)
            nc.tensor.matmul(agg1, id33, gmT[:, s1],
                             start=True, stop=False)
            # hp1 embedded epilogue schedule for half 0 (spread into small
            # sub-blobs so the DVE backlog never starves the exp->PV chain)
            epi_sched = {} if hp == 0 else {
                5: [lambda: epi_uda(0)],
                6: [lambda: epi_stats(0, 0, 4)],
                8: [lambda: epi_stats(0, 4, 8)],
                10: [lambda: epi_rstd(0)],
                13: [lambda: epi_xn(0, 0, 4)],
                15: [lambda: epi_xn(0, 4, 8), lambda: epi_out(0)],
            }
            pv_q = []
            for pp in range(NPAIR):
                co = 256 * pp
                ko_lo = okS[0:E, co:co + P]
                ko_hi = okS[E:P, co + P:co + 2 * P]
                st0 = ps.tile([P, 2, GW], F32, tag="st", name=f"st{hp}_{pp}_0")
                st1 = ps.tile([P, 2, GW], F32, tag="st", name=f"st{hp}_{pp}_1")
                nc.tensor.matmul(st0[:, 0, :], ko_lo, qT[0:E, s0],
                                 start=True, stop=True)
                nc.tensor.matmul(st0[:, 1, :], ko_hi, qT[E:P, s0],
                                 start=True, stop=True)
                nc.tensor.matmul(st1[:, 0, :], ko_lo, qT[0:E, s1],
                                 start=True, stop=True)
                nc.tensor.matmul(st1[:, 1, :], ko_hi, qT[E:P, s1],
                                 start=True, stop=True)
                # PV pipelined TWO pairs deep: exp(p) has ~2 pair-times of
                # slack before the PE would ever wait on it.
                if len(pv_q) >= 2:
                    qq, qt0, qt1 = pv_q.pop(0)
                    go = gob8[:, qq, :, :]
                    nc.tensor.matmul(agg0, go, qt0, start=False, stop=False,
                                     perf_mode=DR)
                    nc.tensor.matmul(agg1, go, qt1, start=False, stop=False,
                                     perf_mode=DR)
                pt0 = sb_pt.tile([P, 2, GW], FP8E5, tag="pt",
                                 name=f"pt{hp}_{pp}_0")
                pt1 = sb_pt.tile([P, 2, GW], FP8E5, tag="pt",
                                 name=f"pt{hp}_{pp}_1")
                # exp split: mostly 1:1 ACT/DVE with a few ACT "bonus"
                # pairs to offset DVE's drip (hp0) / epilogue (hp1) load.
                bonus = pp in ((10, 21) if hp == 0 else (6, 9, 12))
                exp_unit(st0, pt0, "a")
                exp_unit(st1, pt1, "a" if bonus else "v")
                pv_q.append((pp, pt0, pt1))
                if hp == 0:
                    for fn in drip.get(pp, ()):
                        fn()
                else:
                    for fn in epi_sched.get(pp, ()):
                        fn()
            while pv_q:
                qq, qt0, qt1 = pv_q.pop(0)
                go = gob8[:, qq, :, :]
                last = not pv_q
                nc.tensor.matmul(agg0, go, qt0, start=False, stop=last,
                                 perf_mode=DR)
                nc.tensor.matmul(agg1, go, qt1, start=False, stop=last,
                                 perf_mode=DR)
            agg_flush(g0, agg0, eng="v")
            agg_flush(g1, agg1, eng="a")

        # ---------------- epilogue tail (half 1) ----------
        epi_half(1)
        epi_rstd(1)
        epi_xn(1, 0, 4)
        epi_out(1, 0, 1)     # gpsimd gabe + DMA overlap the next xn
        epi_xn(1, 4, 8)
        epi_out(1, 1, 2)

        if dbg is not None:
            nc.sync.dma_start(dbg["qT"], qT)
            nc.sync.dma_start(dbg["gmT"], gmT)
            nc.sync.dma_start(dbg["ags"], ags)
            nc.sync.dma_start(dbg["okS"], okS)
            nc.sync.dma_start(dbg["gob8"],
                              gob8.rearrange("p a b c -> p (a b c)"))
            nc.sync.dma_start(dbg["out_pre"],
                              out_pre.rearrange("p a b -> p (a b)"))
            nc.sync.dma_start(dbg["mvC"], mvC.rearrange("p a b -> p (a b)"))


_CACHED = None


def _build(debug=False):
    global _CACHED
    if _CACHED is not None and not debug:
        return _CACHED
    nc = bacc.Bacc("TRN2", target_bir_lowering=False, debug=False)

    def din(name, shape, dt=F32):
        return nc.dram_tensor(name, shape, dt, kind="ExternalInput").ap()

    map_rows_d = din("map_rows", [NS, E])
    mapT_d = din("mapT", [E, NS], BF16)
    obsT_d = din("obsT", [E, NO], BF16)
    wpb_d = din("wpb", [E, BW], BF16)
    wpf_d = din("wpf", [E, FW], F32R)
    vec_d = din("vpack", [3 * E + 1])
    wup_d = din("wup", [P, GW], BF16)
    out_d = nc.dram_tensor("out", [NS, E], F32, kind="ExternalOutput").ap()

    dbg = None
    if debug:
        def dout(name, shape, dt=F32):
            return nc.dram_tensor(name, shape, dt, kind="ExternalOutput").ap()
        dbg = {
            "qT": dout("dbg_qT", [P, NS], BF16),
            "gmT": dout("dbg_gmT", [H + 1, NS], BF16),
            "ags": dout("dbg_ags", [H + 1, NS], F32R),
            "okS": dout("dbg_okS", [P, NO], BF16),
            "gob8": dout("dbg_gob8", [P, NPAIR * 2 * MPAD], FP8),
            "out_pre": dout("dbg_out_pre", [P, NT * E]),
            "mvC": dout("dbg_mvC", [P, NT * 2]),
        }

    with tile.TileContext(nc) as tc:
        _emit(tc, out_d, map_rows_d, mapT_d, obsT_d, wpb_d, wpf_d,
              vec_d, wup_d, dbg=dbg)
    nc.compile()
    if not debug:
        _CACHED = nc
    return nc


def _prep_in_maps(map_code, obs_code, Wq, Wk, Wv, Wo, bo, gamma, beta):
    f = np.float32
    map_code = np.ascontiguousarray(np.asarray(map_code, dtype=f))
    obs_code = np.asarray(obs_code, dtype=f)

    bf16_np = mybir.dt.np(BF16)

    def to_bf16(x):
        return np.ascontiguousarray(np.asarray(x, dtype=f).astype(bf16_np))

    obsT = np.ascontiguousarray(obs_code.T)

    wq2 = np.concatenate([np.asarray(Wq, f), np.asarray(Wq, f)], axis=1)
    wk2 = np.concatenate([np.asarray(Wk, f), np.asarray(Wk, f)], axis=1)
    wpb = np.zeros((E, BW), dtype=f)
    wpb[:, _WQ0:_WQ0 + 128] = wq2
    wpb[:, _WK0:_WK0 + 128] = wk2
    wpb[:, _WV0:_WV0 + E] = np.asarray(Wv, f)
    wpb[:, _ONES0] = 1.0
    for k in range(H + 1):
        wpb[k, _IDO + k] = 1.0   # identity seed stationary [33, MPAD]

    woe = np.zeros((E, FW), dtype=f)
    woe[0:H, 0:E] = np.asarray(Wo, dtype=f)
    woe[H, E] = 1.0

    vpack = np.concatenate([
        np.asarray(bo, dtype=f), np.asarray(gamma, dtype=f),
        np.asarray(beta, dtype=f), np.full((1,), -SHIFT, dtype=f),
    ])
    shared = {
        "obsT": to_bf16(obsT),
        "wpb": to_bf16(wpb),
        "wpf": np.ascontiguousarray(woe),
        "vpack": np.ascontiguousarray(vpack),
        "wup": np.full((P, GW), 0.125, dtype=bf16_np),
    }
    in_maps = []
    for i in range(NCORES):
        shard = map_code[i * NS:(i + 1) * NS]
        m = dict(shared)
        m["map_rows"] = shard
        m["mapT"] = to_bf16(np.ascontiguousarray(shard.T))
        in_maps.append(m)
    return in_maps


def run(trace=False, **inputs):
    nc = _build()
    in_maps = _prep_in_maps(**inputs)
    res = run_bass_kernel_spmd(nc, in_maps, list(range(NCORES)), trace=trace)
    out = np.concatenate([res.results[i]["out"] for i in range(NCORES)], axis=0)
    return out, res


def kernel(**inputs):
    out, _ = run(trace=False, **inputs)
    return out
